# revision 60
# baseline (speedup 1.0000x reference)
"""DeepseekV2 MLA attention (weight-absorbed, MQA-style latent) on 8 TRN2 NeuronCores.

Sharding: data-parallel over batch (B=2) x tensor-parallel over heads (4 heads/core).
Each core computes, for its batch element and its 4 heads, the partial o_proj
output out_t = [HID, S] (transposed layout); the host sums the 4 partials per
batch element and transposes back.

Mixed-precision design.  The only fast PE mode is fp8e4m3 with DoubleRow
(2x128-deep contraction per instruction at 0.5 cycles/row), so:
  * The large projections (q, ckv, o_proj) run as 3-term fp8 DoubleRow
    products with host-side fp8 residual splits (x = x8 + r8):
    x8*y8 + x8*ry + rx*y8 (~0.1-0.5% error at 1.5x plain-fp8 cost).
  * Scores are 3 DoubleRows per 128-row t-tile: 2 cover the 512-dim latent
    (fp8 ckvT8 x fp8 q_lat; noise is damped through the softmax), the 3rd
    covers rope via packed planes: stationary plane4 = [kp8; kp8], plane5 =
    [kp_res; 0]; moving plane4 = [qp8; qp_res], plane5 = [qp8; *] - summing
    to kp*qp to second order.  Causal-mask adds ride a small fp8 identity
    matmul on the diagonal pairs only.
  * Pass 1 absorbs v_b into the latents: vT16[t,d] = (ckv_norm @ vb)^T per
    head, so the PV matmul is ONE fp16 matmul per t-tile into a single psum
    bank per head (no ol_ps chain, no per-head v_b expansion in pass 2).
  * Row-sums use 128-row ones-matmuls (fp16 for diagonal pairs, fp8
    DoubleRow over a DVE-copied fp8 ex for history pairs), so the softmax
    denominator needs only a DVE reciprocal - no PE broadcast.

Schedule: pass 1 runs mb-major blocks (kg-major for the DMA-bound chunk 0
with quarter-split streams) with terms ordered (0,2,1) so the residual
streams arrive late; the RMSNorm/rope/fp8 chains are emitted inline
(Act/DVE only) and the vT matmuls are deferred one chunk so the PE never
waits on them.  Pass 2 processes chunks 3..0: per chunk, the q k-loop runs
qa-block first (its DVE rope chain hides under the qn blocks), scores
accumulate rope-first, the score->PV pipeline runs 4 pairs deep, and the
previous chunk's o_proj is emitted in ht-quarters after each head to fill
pipeline bubbles.  Output stores alternate sync/scalar/gpsimd queues.

Scale ledger (log2 of stored/true): hid8 0 | wq/wkv/wo +5 | kb16 +3 |
qn16 +2 | ql8 +5 | qpr/qp8 +5 | kp16/kp8 0 (k-rope tables pre-divided by
32) | ckv latents 0 (RMSNorm cancels +5; eps pre-scaled 2^10) | scores +5
(exp scale SCALE/32; masks are fp8 -234 via a x128 fp8 identity matmul) |
ex 0 | vT16 +5 | vo8a/vor8 +5 normalized | oo_ps +10 -> out x 2^-10.

Softmax is max-free (score magnitudes are small; verified on host).
"""
import sys

for _p in ("/opt/trn_rl_repo", "/root/.axon_site/_ro/trn_rl_repo"):
    if _p not in sys.path:
        sys.path.insert(0, _p)

import numpy as np
import ml_dtypes

B, S, HID = 2, 2048, 2048
H, DN, DR, KVR, DV = 16, 128, 64, 512, 128
THETA, EPS = 10000.0, 1e-6
SCALE = float((DN + DR) ** -0.5)
NCORES, HL = 8, 4  # 2 (batch) x 4 (head groups of 4)
CH = 512           # s-chunk width (= psum bank width in fp32)
WS = 32.0          # host-side weight pre-scale (2^5)

FP8 = ml_dtypes.float8_e4m3
BF16 = ml_dtypes.bfloat16


def build_nc(s=S, hid=HID, reps=1):
    import concourse.bacc as bacc
    import concourse.mybir as mybir
    from concourse import tile

    f32 = mybir.dt.float32
    f32r = mybir.dt.float32r
    fp8 = mybir.dt.float8e4
    fp16 = mybir.dt.float16
    bf16 = mybir.dt.bfloat16
    Exp = mybir.ActivationFunctionType.Exp
    Sqrt = mybir.ActivationFunctionType.Sqrt
    Copy = mybir.ActivationFunctionType.Copy
    mult = mybir.AluOpType.mult
    subtract = mybir.AluOpType.subtract
    DRow = mybir.MatmulPerfMode.DoubleRow

    def r(ap):
        return ap.bitcast(f32r)

    NCH = s // CH      # s-chunks
    KT = hid // 128    # contraction tiles over HID
    KG = KT // 2       # DoubleRow contraction pairs
    NT = s // 128      # t-tiles
    NPR = NT // 2      # t-tile pairs

    nc = bacc.Bacc("TRN2", target_bir_lowering=False, debug=False,
                   enable_asserts=False, num_devices=NCORES)

    hid_d = nc.dram_tensor("hid8", [hid, s], fp8, kind="ExternalInput").ap()
    hidr_d = nc.dram_tensor("hidr8", [hid, s], fp8, kind="ExternalInput").ap()
    wq_d = nc.dram_tensor("wq8", [hid, 768], fp8, kind="ExternalInput").ap()
    wqr_d = nc.dram_tensor("wqr8", [hid, 768], fp8, kind="ExternalInput").ap()
    wkv_d = nc.dram_tensor("wkv8", [hid, KVR + 2 * DR], fp8, kind="ExternalInput").ap()
    wkvr_d = nc.dram_tensor("wkvr8", [hid, KVR + 2 * DR], fp8, kind="ExternalInput").ap()
    ln_d = nc.dram_tensor("ln_t", [128, 4], f32, kind="ExternalInput").ap()
    kb_d = nc.dram_tensor("kb16", [HL, DN, KVR], fp16, kind="ExternalInput").ap()
    vb_d = nc.dram_tensor("vb16_t", [HL, KVR, DV], fp16, kind="ExternalInput").ap()
    wo_d = nc.dram_tensor("wo8_t", [HL * DV, hid], fp8, kind="ExternalInput").ap()
    wor_d = nc.dram_tensor("wor8_t", [HL * DV, hid], fp8, kind="ExternalInput").ap()
    cs2_d = nc.dram_tensor("cs2", [128, s], bf16, kind="ExternalInput").ap()
    ckr_d = nc.dram_tensor("ckrope", [128, s], bf16, kind="ExternalInput").ap()
    mask_d = nc.dram_tensor("masks", [128, 4, CH], fp8, kind="ExternalInput").ap()
    idm_d = nc.dram_tensor("identm8", [128, 128], fp8, kind="ExternalInput").ap()
    onesq_d = nc.dram_tensor("ones_sq", [128, 128], f32r, kind="ExternalInput").ap()
    on16_d = nc.dram_tensor("ones16", [128, 128], fp16, kind="ExternalInput").ap()
    on8_d = nc.dram_tensor("ones8", [128, 2, 128], fp8, kind="ExternalInput").ap()
    out_d = nc.dram_tensor("out_t", [hid, s], fp16, kind="ExternalOutput").ap()

    with tile.TileContext(nc) as tc, \
         nc.allow_low_precision(reason="fp8/fp16 matmuls; psum accum stays fp32"):
        with tc.tile_pool(name="res", bufs=1) as res, \
             tc.tile_pool(name="psp", bufs=8, space="PSUM") as psp:

            def ps_tile(name):
                return psp.tile([128, CH], f32, tag="ps", name=name)

            # resident tiles
            hid8 = res.tile([128, KT, s], fp8, name="hid8_sb")
            wq8 = res.tile([128, KT, 768], fp8, name="wq8_sb")
            wqr = res.tile([128, KT, 768], fp8, name="wqr_sb")
            wkv8 = res.tile([128, KT, KVR + 2 * DR], fp8, name="wkv8_sb")
            wkvr = res.tile([128, KT, KVR + 2 * DR], fp8, name="wkvr_sb")
            wo8 = res.tile([128, HL, hid], fp8, name="wo8_sb")
            wor = res.tile([128, HL, hid], fp8, name="wor_sb")
            ckvT8 = res.tile([128, 6, s], fp8, name="ckvT8")
            vT16 = res.tile([128, HL, NT, DV], fp16, name="vT16")
            kb16 = res.tile([128, HL, KVR], fp16, name="kb16_sb")
            vb16 = res.tile([128, HL, 4, DV], fp16, name="vb16_sb")
            cs2 = res.tile([128, s], bf16, name="cs2_sb")
            ckr = res.tile([128, s], bf16, name="ckr_sb")
            masks = res.tile([128, 4, CH], fp8, name="masks_sb")
            idm8 = res.tile([128, 128], fp8, name="idm8_sb")
            ln_sb = res.tile([128, 4], f32, name="ln_sb")
            onesq = res.tile([128, 128], f32r, name="onesq_sb")
            on16 = res.tile([128, 128], fp16, name="on16_sb")
            on8 = res.tile([128, 2, 128], fp8, name="on8_sb")
            zb128 = res.tile([128, 1], f32, name="zb128")
            epsb = res.tile([128, 1], f32, name="epsb")
            c_one = res.tile([128, 1], f32, name="c_one")
            cm1 = res.tile([128, 1], f32, name="cm1")
            scr1 = res.tile([1, 4], f32, name="scr1")
            nc.vector.memset(zb128[:], 0.0)
            nc.vector.memset(epsb[:], EPS * WS * WS)
            nc.vector.memset(c_one[:], 1.0)
            nc.vector.memset(cm1[:], -1.0)
            # plane 5 of ckvT8 is an all-zero DoubleRow partner for the rope
            # plane (4): contributes 0 to the score accumulation
            nc.vector.memset(ckvT8[:, 5, :], 0.0)
            # touch every activation function once now so the table loads
            # (1.3us each) hide under the startup DMAs instead of stalling
            # pass 2's first Exp
            nc.scalar.activation(scr1[0:1, 0:1], zb128[0:1, 0:1], Exp)
            nc.scalar.activation(scr1[0:1, 1:2], zb128[0:1, 0:1], Sqrt)
            nc.scalar.activation(scr1[0:1, 2:3], zb128[0:1, 0:1], Copy)

            # DMA queue assignment: sync/scalar carry the pass-1-critical loads
            # (first matmul needs wkv8 + hid8[j0] + hidr[j0]); pass-2-only
            # residents ride the pool queue (Pool engine has slack; SWDGE
            # desc-gen costs ~1us of Pool ENGINE time per copy).
            # first-needed loads split into k-quarters so chunk 0 can start
            # after quarter-transfers on the serial DMA engines
            for kh in range(4):
                ks = slice(kh * (KT // 4), (kh + 1) * (KT // 4))
                rs_ = slice(kh * (hid // 4), (kh + 1) * (hid // 4))
                nc.sync.dma_start(wkv8[:, ks, :],
                                  wkv_d[rs_, :].rearrange("(g p) n -> p g n", p=128))
                nc.scalar.dma_start(hid8[:, ks, 0:CH],
                                    hid_d[rs_, 0:CH].rearrange("(g p) t -> p g t", p=128))
                nc.scalar.dma_start(wkvr[:, ks, :],
                                    wkvr_d[rs_, :].rearrange("(g p) n -> p g n", p=128))
            if NCH > 1:
                nc.scalar.dma_start(hid8[:, :, CH:2 * CH],
                                    hid_d[:, CH:2 * CH].rearrange("(g p) t -> p g t", p=128))
            # post-0 consts before the later hid chunks (those have slack)
            nc.scalar.dma_start(ckr[:], ckr_d)
            nc.scalar.dma_start(ln_sb[:], ln_d)
            nc.scalar.dma_start(onesq[:], onesq_d)
            # vb16 is needed by pass-1 post(0) (v^T precompute); the gpsimd
            # queue is otherwise idle until the gated pass-2 loads
            nc.gpsimd.dma_start(vb16[:], vb_d.rearrange("h (ci p) d -> p h ci d", p=128))
            for j in range(2, NCH):
                sl = slice(j * CH, (j + 1) * CH)
                nc.scalar.dma_start(hid8[:, :, sl],
                                    hid_d[:, sl].rearrange("(g p) t -> p g t", p=128))
            def load_pass2_weights():
                # deferred: these ride the DMA engines behind the pass-1
                # critical streams (hid/hidr per chunk), not ahead of them
                nc.gpsimd.dma_start(cs2[:], cs2_d)
                nc.gpsimd.dma_start(wq8[:], wq_d.rearrange("(g p) n -> p g n", p=128))
                nc.gpsimd.dma_start(wqr[:], wqr_d.rearrange("(g p) n -> p g n", p=128))
                nc.gpsimd.dma_start(kb16[:], kb_d.rearrange("h d c -> d h c"))
                nc.gpsimd.dma_start(masks[:], mask_d)
                nc.gpsimd.dma_start(idm8[:], idm_d)
                nc.gpsimd.dma_start(on16[:], on16_d)
                nc.gpsimd.dma_start(on8[:], on8_d)
                nc.gpsimd.dma_start(wo8[:], wo_d.rearrange("(a p) n -> p a n", p=128))
                nc.gpsimd.dma_start(wor[:], wor_d.rearrange("(a p) n -> p a n", p=128))

            for _rep in range(reps):
              # ---------------- pass 1: latent KV (ckvT8, ckvN8+r, k_pe rot) --
              with tc.tile_pool(name="p1", bufs=1) as p1:
                prev_vt = None
                for j in range(NCH):
                    sl = slice(j * CH, (j + 1) * CH)
                    hidr = p1.tile([128, KT, CH], fp8, tag="hidr", bufs=2, name="hidr")
                    nc.sync.dma_start(hidr[:],
                                      hidr_d[:, sl].rearrange("(g p) t -> p g t", p=128))
                    if j == NCH - 1:
                        # WAW-gate the big pass-2 loads behind the last
                        # critical pass-1 stream: the scheduler orders by data
                        # deps, so dep-free DMAs would otherwise hog the
                        # serial DMA engine ahead of the hid/hidr streams
                        for big in (wq8, wqr, wo8, wor):
                            nc.gpsimd.tensor_copy(big[0:1, 0, 0:1], hidr[0:1, 0, 0:1])
                        nc.gpsimd.tensor_copy(cs2[0:1, 0:1], hidr[0:1, 0, 0:1])
                        load_pass2_weights()
                    cps = [ps_tile(f"cps{ci}") for ci in range(4)]
                    kp_ps = ps_tile("kp_ps")
                    nmb = (KVR + 2 * DR) // 128
                    # mb-major blocks: each output block's accumulation closes
                    # early, so its evac + RMSNorm square overlap later blocks
                    # and the staggered var matmuls never wait on the DVE.
                    # Within a block, terms run (0, 2, 1) so the hidr stream
                    # is not needed until 2/3 through the block.
                    c_sb = [None] * 4
                    sq_box = [None] * 4
                    kp_box = []
                    var_box = []
                    def close_block(mb):
                        if mb < 4:
                            t = p1.tile([128, CH], f32r, tag="c_sb", bufs=5,
                                        name=f"c_sb{mb}")
                            nc.scalar.copy(t[:], cps[mb][:])
                            c_sb[mb] = t
                            sq = p1.tile([128, CH], f32r, tag="sqt", bufs=2, name="sqt")
                            nc.vector.tensor_mul(sq[:], t[:], t[:])
                            sq_box[mb] = sq
                        else:
                            kp = p1.tile([128, CH], f32, tag="kp_sb", bufs=2,
                                         name="kp_sb")
                            nc.scalar.copy(kp[:], kp_ps[:])
                            kp_box.append(kp)

                    if False:
                        # chunk 0 is DMA-startup-bound: term-major gives the
                        # hidr/wkvr streams the longest lead time
                        for ti, term in enumerate((0, 2, 1)):
                            for kg in range(KG):
                                st_ = (ti == 0 and kg == 0)
                                sp_ = (ti == 2 and kg == KG - 1)
                                kk = slice(2 * kg, 2 * kg + 2)
                                w_t, h_t = ((wkv8[:, kk, :], hid8[:, kk, sl]),
                                            (wkv8[:, kk, :], hidr[:, kk, :]),
                                            (wkvr[:, kk, :], hid8[:, kk, sl]))[term]
                                for mb in range(nmb):
                                    mbs = slice(mb * 128, (mb + 1) * 128)
                                    out = cps[mb][:] if mb < 4 else kp_ps[:]
                                    nc.tensor.matmul(out, w_t[:, :, mbs], h_t,
                                                     start=st_, stop=sp_, perf_mode=DRow)
                        for mb in range(nmb):
                            close_block(mb)
                    else:
                        for mb in range(nmb):
                            mbs = slice(mb * 128, (mb + 1) * 128)
                            out = cps[mb][:] if mb < 4 else kp_ps[:]
                            for ti, term in enumerate((0, 2, 1)):
                                for kg in range(KG):
                                    st_ = (ti == 0 and kg == 0)
                                    sp_ = (ti == 2 and kg == KG - 1)
                                    kk = slice(2 * kg, 2 * kg + 2)
                                    w_t, h_t = ((wkv8[:, kk, :], hid8[:, kk, sl]),
                                                (wkv8[:, kk, :], hidr[:, kk, :]),
                                                (wkvr[:, kk, :], hid8[:, kk, sl]))[term]
                                    nc.tensor.matmul(out, w_t[:, :, mbs], h_t,
                                                     start=st_, stop=sp_, perf_mode=DRow)
                            close_block(mb)
                    var_box.append(ps_tile("var_ps"))
                    for ci in range(4):
                        nc.tensor.matmul(var_box[0][:], onesq[:], r(sq_box[ci][:]),
                                         start=(ci == 0), stop=(ci == 3))
                    var_ps = var_box[0]

                    # normalization chain (Act/DVE only; PE moves on to the
                    # next chunk's k-loop meanwhile)
                    sdf = p1.tile([128, CH], f32, tag="sdf", bufs=1, name="sdf")
                    nc.scalar.activation(sdf[:], var_ps[:], Sqrt, bias=epsb[:],
                                         scale=1.0 / KVR)
                    rsq = p1.tile([128, CH], f32r, tag="rsq", bufs=1, name="rsq")
                    nc.vector.reciprocal(rsq[:], sdf[:])
                    ckvT16 = p1.tile([128, 4, CH], fp16, tag="ckvT16", bufs=2,
                                     name="ckvT16")
                    for ci in range(4):
                        nc.vector.scalar_tensor_tensor(ckvT16[:, ci, :], c_sb[ci][:],
                                                       ln_sb[:, ci:ci + 1], rsq[:],
                                                       op0=mult, op1=mult)
                        nc.scalar.copy(ckvT8[:, ci, sl], ckvT16[:, ci, :])
                    # k_pe rope: rows 0:64 = a*cos/WS, 64:128 = b*sin/WS.
                    # Products are written to base-0 slabs (partition shift
                    # rides the psum-input ops), then added partition-aligned.
                    ta_s = p1.tile([32, 2, CH], f32, tag="ta_s", bufs=1, name="ta_s")
                    tb_s = p1.tile([32, 2, CH], f32, tag="tb_s", bufs=1, name="tb_s")
                    kp16 = p1.tile([64, CH], fp16, tag="kp16", bufs=2, name="kp16")
                    for i2 in range(2):
                        nc.vector.tensor_mul(ta_s[:, i2, :],
                                             kp_box[0][32 * i2:32 * i2 + 32, :],
                                             ckr[32 * i2:32 * i2 + 32, sl])
                        nc.vector.tensor_mul(tb_s[:, i2, :],
                                             kp_box[0][64 + 32 * i2:96 + 32 * i2, :],
                                             ckr[64 + 32 * i2:96 + 32 * i2, sl])
                        nc.vector.tensor_add(kp16[32 * i2:32 * i2 + 32, :],
                                             ta_s[:, i2, :], tb_s[:, i2, :])
                    # plane4 = [kp8; kp8], plane5 rows 0:64 = kp residual
                    # (rows 64:128 stay zero): with moving planes
                    # [qp8; qp_res] / [qp8; *] the three products sum to
                    # kp8*qp + kp_r*qp8 ~ kp*qp to second order
                    nc.scalar.activation(ckvT8[0:64, 4, sl], kp16[:], Copy,
                                         scale=1.0)
                    nc.scalar.activation(ckvT8[64:128, 4, sl], kp16[:], Copy,
                                         scale=1.0)
                    nc.vector.scalar_tensor_tensor(ckvT8[0:64, 5, sl], kp16[:],
                                                   c_one[0:64, :],
                                                   ckvT8[0:64, 4, sl],
                                                   op0=mult, op1=subtract)

                    # v^T[t, d] = ckv^T.T @ vb per head (weight-absorbed value
                    # expansion; PV then needs 1 matmul per t-tile).  The PE
                    # matmuls are deferred one chunk so they never wait on the
                    # freshly-written ckvT16.
                    def make_vt(j, ckvT16):
                        def vt():
                            vt_box = [ps_tile(f"vt_ps{h}") for h in range(HL)]
                            for h in range(HL):
                                for q in range(4):
                                    lb = slice(q * 128, (q + 1) * 128)
                                    for ci in range(4):
                                        nc.tensor.matmul(
                                            vt_box[h][:, q * 128:(q + 1) * 128],
                                            ckvT16[:, ci, lb], vb16[:, h, ci, :],
                                            start=(ci == 0), stop=(ci == 3))
                            for h in range(HL):
                                nc.scalar.copy(vT16[:, h, 4 * j:4 * j + 4, :],
                                               vt_box[h][:])
                        return vt

                    vt_j = make_vt(j, ckvT16)
                    if prev_vt is not None:
                        prev_vt()
                    prev_vt = vt_j
                prev_vt()

              # ---------------- pass 2: q proj + attention + o_proj -----------
              with tc.tile_pool(name="p2", bufs=1) as p2:
                prev_oproj = None
                for jo, j in enumerate(range(NCH - 1, -1, -1)):
                    sl = slice(j * CH, (j + 1) * CH)

                    hidr2 = p2.tile([128, KT, CH], fp8, tag="hidr2", bufs=1, name="hidr2")
                    nc.sync.dma_start(hidr2[:],
                                      hidr_d[:, sl].rearrange("(g p) t -> p g t", p=128))
                    qn_ps = [ps_tile(f"qn_ps{h}") for h in range(HL)]
                    qa_ps = [ps_tile(f"qa_ps{p}") for p in range(2)]
                    ql8a = p2.tile([128, 6, HL, CH], fp8, tag="ql8a", bufs=2, name="ql8a")
                    if jo < 2:
                        # plane 5 is a dead DoubleRow partner (zero stationary);
                        # it just has to hold valid fp8 bits, so clear only the
                        # first two ring allocations
                        nc.gpsimd.memset(ql8a[:, 5, :, :], 0.0)
                    qpr16 = p2.tile([64, HL, CH], fp16, tag="qpr16", bufs=1, name="qpr16")
                    qn16 = []
                    # block-major: qa (rope q) first so its DVE rope chain runs
                    # under the qn blocks; terms (0, 2, 1) so hidr2 is needed
                    # only 2/3 into each block; each qn head evacs right after
                    # its block closes
                    def qblock(outs, cols):
                        for ti, term in enumerate((0, 2, 1)):
                            w_t = wq8 if term in (0, 1) else wqr
                            h_t = hidr2[:] if term == 1 else hid8[:, :, sl]
                            for kg in range(KG):
                                st_ = (ti == 0 and kg == 0)
                                sp_ = (ti == 2 and kg == KG - 1)
                                kk = slice(2 * kg, 2 * kg + 2)
                                for out, cb in zip(outs, cols):
                                    nc.tensor.matmul(out[:], w_t[:, kk, cb],
                                                     h_t[:, kk, :], start=st_,
                                                     stop=sp_, perf_mode=DRow)
                    qblock(qa_ps, [slice(512 + p * 128, 512 + (p + 1) * 128)
                                   for p in range(2)])
                    for p in range(2):
                        # q rope: qc = qa*cos; qr = rotate_half(qa)*sin with the
                        # sign flip folded into an stt (cross-partition reads)
                        qc = p2.tile([128, CH], bf16, tag="qc", bufs=1, name="qc")
                        qr = p2.tile([128, CH], bf16, tag="qr", bufs=1, name="qr")
                        for hh in (0, 64):
                            nc.vector.tensor_mul(qc[hh:hh + 64, :], qa_ps[p][hh:hh + 64, :],
                                                 cs2[0:64, sl])
                            nc.vector.scalar_tensor_tensor(qr[hh:hh + 32, :],
                                                           qa_ps[p][hh + 32:hh + 64, :],
                                                           cm1[64:96, :], cs2[64:96, sl],
                                                           op0=mult, op1=mult)
                            nc.vector.tensor_mul(qr[hh + 32:hh + 64, :],
                                                 qa_ps[p][hh:hh + 32, :], cs2[96:128, sl])
                        for i, hh in ((0, 0), (1, 64)):
                            h2 = 2 * p + i
                            nc.vector.tensor_add(qpr16[0:32, h2, :],
                                                 qc[hh:hh + 32, :], qr[hh:hh + 32, :])
                            nc.vector.tensor_add(qpr16[32:64, h2, :],
                                                 qc[hh + 32:hh + 64, :], qr[hh + 32:hh + 64, :])
                    # rope q in fp8: plane4 = [qp8; qp_res], plane5 rows 0:64 =
                    # qp8 again (pairs with the k-side residual)
                    for h in range(HL):
                        nc.scalar.activation(ql8a[0:64, 4, h, :], qpr16[:, h, :],
                                             Copy, scale=1.0)
                        nc.vector.scalar_tensor_tensor(ql8a[64:128, 4, h, :],
                                                       qpr16[:, h, :], c_one[0:64, :],
                                                       ql8a[0:64, 4, h, :],
                                                       op0=mult, op1=subtract)
                        nc.scalar.activation(ql8a[0:64, 5, h, :], qpr16[:, h, :],
                                             Copy, scale=1.0)
                    for h in range(HL):
                        qblock([qn_ps[h]], [slice(h * 128, (h + 1) * 128)])
                        t = p2.tile([128, CH], fp16, tag="qn16", bufs=4, name=f"qn16_{h}")
                        nc.scalar.activation(t[:], qn_ps[h][:], Copy, scale=0.125)
                        qn16.append(t)

                    # previous chunk's o_proj: one ht-quarter is emitted
                    # after each head below, peppering the PE queue so o_proj
                    # matmuls fill the attention pipeline bubbles
                    oproj_quarters = prev_oproj if prev_oproj is not None else []
                    prev_oproj = None

                    vo8a = p2.tile([128, HL, CH], fp8, tag="vo8a", bufs=2, name="vo8a")
                    vor8 = p2.tile([128, HL, CH], fp8, tag="vor8", bufs=2, name="vor8")
                    prev_tail = None
                    for h in range(HL):
                        # q_lat^T[c, s]: plain fp8 matmuls (K=128), evac x 2^-2
                        for ci in range(4):
                            ql_ps = ps_tile("ql_ps")
                            nc.tensor.matmul(ql_ps[:], kb16[:, h, ci * 128:(ci + 1) * 128],
                                             qn16[h][:], start=True, stop=True)
                            if ci % 2 == 0:
                                nc.scalar.activation(ql8a[:, ci, h, :], ql_ps[:], Copy,
                                                     scale=1.0)
                            else:
                                nc.vector.tensor_scalar_mul(ql8a[:, ci, h, :], ql_ps[:], 1.0)

                        # emit the previous head's tail now so its psum-freeing
                        # chain overlaps this head's ql/score matmuls
                        if prev_tail is not None:
                            prev_tail()
                            prev_tail = None

                        # t-pair order: diagonal pairs first, then history pairs
                        prs = [(2 * j, (0, 0), True), (2 * j + 1, (256, 384), True)] + \
                              [(m, (0, 0), False) for m in range(0, 2 * j)]

                        def do_pair(m, sts, diag):
                            e8p = None if diag else p2.tile([128, 2, CH], fp8,
                                                            tag="e8p", bufs=5, name="e8p")
                            exs = []
                            for par in range(2):
                                st = sts[par]
                                t_i = 2 * m + par
                                tb = slice(t_i * 128, (t_i + 1) * 128)
                                sc_ps = ps_tile("sc_ps")
                                nc.tensor.matmul(sc_ps[:, st:], ckvT8[:, 4:6, tb],
                                                 ql8a[:, 4:6, h, st:],
                                                 start=True, stop=False, perf_mode=DRow)
                                nc.tensor.matmul(sc_ps[:, st:], ckvT8[:, 0:2, tb],
                                                 ql8a[:, 0:2, h, st:],
                                                 start=False, stop=False, perf_mode=DRow)
                                if diag:
                                    # mask add as a tiny fp16 identity-matmul on
                                    # the PE, folded into the score accumulation
                                    kd = t_i - 4 * j
                                    ma, mb2 = ((0, 128), (0, 256),
                                               (256, 384), (384, 512))[kd]
                                    nc.tensor.matmul(sc_ps[:, ma:mb2], idm8[:],
                                                     masks[:, kd, ma:mb2],
                                                     start=False, stop=False)
                                nc.tensor.matmul(sc_ps[:, st:], ckvT8[:, 2:4, tb],
                                                 ql8a[:, 2:4, h, st:],
                                                 start=False, stop=True, perf_mode=DRow)
                                ex16 = p2.tile([128, CH], fp16, tag="ex16", bufs=8, name="ex16")
                                nc.scalar.activation(ex16[:, st:], sc_ps[:, st:], Exp,
                                                     bias=zb128[:], scale=SCALE / WS)
                                if e8p is not None:
                                    nc.vector.tensor_copy(e8p[:, par, :], ex16[:])
                                exs.append(ex16)
                            return tuple(exs) + (e8p,)

                        vo_box = []
                        rs_box = []

                        def pv(idx, m, sts, ex_a, ex_b, e8p):
                            first, last = (idx == 0), (idx == len(prs) - 1)
                            for par, ext in ((0, ex_a), (1, ex_b)):
                                st = sts[par]
                                t_i = 2 * m + par
                                nc.tensor.matmul(vo_box[0][:, st:],
                                                 vT16[:, h, t_i, :], ext[:, st:],
                                                 start=(first and par == 0),
                                                 stop=(last and par == 1))
                                if e8p is None:
                                    nc.tensor.matmul(rs_box[0][:, st:], on16[:], ext[:, st:],
                                                     start=(first and par == 0),
                                                     stop=(last and par == 1))
                            if e8p is not None:
                                nc.tensor.matmul(rs_box[0][:, :], on8[:], e8p[:],
                                                 start=first, stop=last, perf_mode=DRow)

                        pend = []
                        for idx, (m, sts, diag) in enumerate(prs):
                            pair_t = do_pair(m, sts, diag)
                            if idx == 0:
                                vo_box.append(ps_tile("vo_ps"))
                                rs_box.append(ps_tile("rs_ps"))
                            pend.append((idx, m, sts) + pair_t)
                            if len(pend) > 4:
                                pv(*pend.pop(0))
                        for pd in pend:
                            pv(*pd)

                        def make_tail(h, vo_ps, rs_ps):
                            def tail():
                                # softmax denominator: full-row reciprocal on DVE
                                rbc = p2.tile([128, CH], f32r, tag="rbc", bufs=1, name="rbc")
                                nc.vector.reciprocal(rbc[:], rs_ps[:])
                                # normalize v-out, fp8 + residual split
                                tmp16 = p2.tile([128, CH], fp16, tag="tmp16", bufs=1, name="tmp16")
                                nc.vector.scalar_tensor_tensor(tmp16[:], vo_ps[:],
                                                               c_one[:], rbc[:],
                                                               op0=mult, op1=mult)
                                nc.gpsimd.tensor_copy(vo8a[:, h, :], tmp16[:])
                                nc.vector.scalar_tensor_tensor(vor8[:, h, :], tmp16[:],
                                                               c_one[:], vo8a[:, h, :],
                                                               op0=mult, op1=subtract)
                            return tail

                        prev_tail = make_tail(h, vo_box[0], rs_box[0])
                        if h < len(oproj_quarters):
                            oproj_quarters[h]()
                    prev_tail()

                    # o_proj partial (3-term fp8x2): out^T = sum_h wo^T.T @ v_out^T
                    def make_oproj(sl, vo8a, vor8):
                        def oproj(hts):
                            for ht in hts:
                                htb = slice(ht * 128, (ht + 1) * 128)
                                oo_ps = ps_tile("oo_ps")
                                for g2 in range(2):
                                    hh2 = slice(2 * g2, 2 * g2 + 2)
                                    nc.tensor.matmul(oo_ps[:], wo8[:, hh2, htb], vo8a[:, hh2, :],
                                                     start=(g2 == 0), stop=False, perf_mode=DRow)
                                    nc.tensor.matmul(oo_ps[:], wo8[:, hh2, htb], vor8[:, hh2, :],
                                                     start=False, stop=False, perf_mode=DRow)
                                    nc.tensor.matmul(oo_ps[:], wor[:, hh2, htb], vo8a[:, hh2, :],
                                                     start=False, stop=(g2 == 1), perf_mode=DRow)
                                oo_sb = p2.tile([128, CH], fp16, tag="oo_sb", bufs=2, name="oo_sb")
                                if ht % 2 == 0:
                                    nc.scalar.activation(oo_sb[:], oo_ps[:], Copy,
                                                         scale=1.0 / (WS * WS))
                                else:
                                    nc.vector.tensor_scalar_mul(oo_sb[:], oo_ps[:],
                                                                1.0 / (WS * WS))
                                if ht % 3 == 0:
                                    nc.sync.dma_start(out_d[htb, sl], oo_sb[:])
                                elif ht % 3 == 1:
                                    nc.scalar.dma_start(out_d[htb, sl], oo_sb[:])
                                else:
                                    nc.gpsimd.dma_start(out_d[htb, sl], oo_sb[:])
                        return oproj

                    _op = make_oproj(sl, vo8a, vor8)
                    prev_oproj = [
                        (lambda q=q, f=_op: f(range(4 * q, 4 * q + 4)))
                        for q in range(4)]
                for q in prev_oproj:
                    q()

    nc.compile()
    return nc


# ---------------------------------------------------------------------------
# host-side input prep / output assembly
# ---------------------------------------------------------------------------
_PERM = np.concatenate([np.arange(0, DR, 2), np.arange(1, DR, 2)])


def _rope_tables(pos, s):
    inv_freq = 1.0 / (THETA ** (np.arange(0, DR, 2, dtype=np.float64) / DR))
    t = pos.astype(np.float64)
    freqs = t[:, None] * inv_freq
    emb = np.concatenate([freqs, freqs], axis=-1)          # [s, DR]
    cosT = np.cos(emb).T.astype(np.float32)                # [DR, s]
    sinT = np.sin(emb).T.astype(np.float32)
    return cosT, sinT


def _masks():
    t = np.arange(128)[:, None]
    c = np.arange(CH)[None, :]
    m = np.zeros((128, 4, CH), np.float32)
    for kd in range(4):
        m[:, kd, :] = np.where(c >= 128 * kd + t, 0.0, -30000.0).astype(np.float32)
    return m


def _fp8_split(x):
    a = x.astype(FP8)
    r = (x - a.astype(np.float32)).astype(FP8)
    return a, r


def prep_core_inputs(inputs, core, s=S, hid=HID):
    b, g = core // 4, core % 4
    heads = slice(HL * g, HL * (g + 1))
    hs = np.asarray(inputs["hidden_states"], np.float32)[b, :s, :hid]
    m = {}
    m["hid8"], m["hidr8"] = _fp8_split(np.ascontiguousarray(hs.T))

    wq = np.asarray(inputs["q_nope_weight"], np.float32).reshape(H, DN, HID)[heads, :, :hid]
    wq_t = wq.transpose(2, 0, 1).reshape(hid, HL * DN)
    wqp = np.asarray(inputs["q_pe_weight"], np.float32).reshape(H, DR, HID)[heads, :, :hid]
    a = wqp[:, _PERM, :]                                   # [4, 64, hid]
    A = a.reshape(2, 128, hid)
    wqpe_t = np.concatenate([A[0], A[1]], axis=0).T
    m["wq8"], m["wqr8"] = _fp8_split(np.concatenate([wq_t, wqpe_t], axis=1) * WS)

    wkv = np.asarray(inputs["kv_a_weight"], np.float32)[:, :hid]
    kpe_a = wkv[KVR:][_PERM]
    kpe_b = np.concatenate([-kpe_a[32:], kpe_a[:32]], axis=0)
    wkv_t = np.ascontiguousarray(
        np.concatenate([wkv[:KVR], kpe_a, kpe_b], axis=0).T * WS)
    m["wkv8"], m["wkvr8"] = _fp8_split(wkv_t)

    m["ln_t"] = np.ascontiguousarray(
        np.asarray(inputs["kv_a_ln_weight"], np.float32).reshape(4, 128).T)
    m["kb16"] = (np.asarray(inputs["k_b_weight"], np.float32)[heads] * (WS / 4)).astype(np.float16)
    m["vb16_t"] = np.ascontiguousarray(
        np.asarray(inputs["v_b_weight"], np.float32)[heads].transpose(0, 2, 1) * WS
    ).astype(np.float16)
    wo_t = np.ascontiguousarray(
        np.asarray(inputs["o_weight"], np.float32)[:hid, HL * DV * g:HL * DV * (g + 1)].T * WS)
    m["wo8_t"], m["wor8_t"] = _fp8_split(wo_t)

    pos = np.asarray(inputs["position_ids"]).reshape(-1)[:s]
    cosT, sinT = _rope_tables(pos, s)                      # [64, s] each
    m["cs2"] = np.ascontiguousarray(np.vstack([cosT, sinT])).astype(BF16)
    m["ckrope"] = np.ascontiguousarray(
        np.vstack([cosT, sinT]) * (1.0 / WS)).astype(BF16)
    m["masks"] = (_masks() / 128.0).astype(FP8)
    m["identm8"] = (np.eye(128, dtype=np.float32) * 128.0).astype(FP8)
    m["ones_sq"] = np.ones((128, 128), np.float32)
    m["ones16"] = np.ones((128, 128), np.float16)
    m["ones8"] = np.ones((128, 2, 128), np.float32).astype(FP8)
    return m


_NC_CACHE = {}


def _get_nc():
    if "nc" not in _NC_CACHE:
        _NC_CACHE["nc"] = build_nc()
    return _NC_CACHE["nc"]


def kernel(**inputs):
    from concourse import bass_utils

    nc = _get_nc()
    in_maps = [prep_core_inputs(inputs, c) for c in range(NCORES)]
    res = bass_utils.run_bass_kernel_spmd(nc, in_maps, core_ids=list(range(NCORES)))
    out = np.empty((B, S, HID), np.float32)
    for b in range(B):
        acc = np.array(res.results[4 * b]["out_t"], np.float32)
        for g in range(1, 4):
            acc += res.results[4 * b + g]["out_t"]
        out[b] = acc.T
    return out



# revision 61
# speedup vs baseline: 1.0226x; 1.0226x over previous
"""DeepseekV2 MLA attention (weight-absorbed, MQA-style latent) on 8 TRN2 NeuronCores.

Sharding: data-parallel over batch (B=2) x tensor-parallel over heads (4 heads/core).
Each core computes, for its batch element and its 4 heads, the partial o_proj
output out_t = [HID, S] (transposed layout); the host sums the 4 partials per
batch element and transposes back.

Mixed-precision design.  The only fast PE mode is fp8e4m3 with DoubleRow
(2x128-deep contraction per instruction at 0.5 cycles/row), so:
  * The large projections (q, ckv, o_proj) run as 3-term fp8 DoubleRow
    products with host-side fp8 residual splits (x = x8 + r8):
    x8*y8 + x8*ry + rx*y8, keeping ~0.1-0.5% error at 1.5x the plain-fp8
    cost (4x cheaper than fp32r would be).
  * The score matmul's latent part is fp8 DoubleRow against fp8-quantized
    ckvT/q_lat (that noise enters through the softmax exponent and is
    damped); the rope part, q_lat (K=128) and causal-mask adds run on the
    PE in fp16/fp8 at 1.0 cycles/row, where fp8 would gain nothing.
  * The PV matmul, its rs row-sum (diagonal pairs) and v_b run in fp16 so
    the exp() output feeds the PV matmul directly with no per-tile
    requantization chain; history-pair row-sums use an fp8 DoubleRow
    ones-matmul over a pool-copied fp8 ex (error ~rs_err/sqrt(history)).

Scale ledger (log2 of stored/true): hid8 0 | wq/wkv/kb/vb/wo +5 | qn16 +2 |
ql8 +5 | qpr16 +5 | kper16 0 (k-rope tables pre-divided by 32) | ckv latents
0 (RMSNorm cancels +5; eps pre-scaled 2^10) | scores +5 (exp scale SCALE/32;
masks are fp8 -234 applied via a x128 fp8 identity matmul) | ex 0 |
ol16 -6 unnormalized | vo8a/vor8 +5 normalized | oo_ps +10 -> out x 2^-10.

Softmax is max-free (score magnitudes are small; verified on host).
"""
import sys

for _p in ("/opt/trn_rl_repo", "/root/.axon_site/_ro/trn_rl_repo"):
    if _p not in sys.path:
        sys.path.insert(0, _p)

import numpy as np
import ml_dtypes

B, S, HID = 2, 2048, 2048
H, DN, DR, KVR, DV = 16, 128, 64, 512, 128
THETA, EPS = 10000.0, 1e-6
SCALE = float((DN + DR) ** -0.5)
NCORES, HL = 8, 4  # 2 (batch) x 4 (head groups of 4)
CH = 512           # s-chunk width (= psum bank width in fp32)
WS = 32.0          # host-side weight pre-scale (2^5)

FP8 = ml_dtypes.float8_e4m3
BF16 = ml_dtypes.bfloat16


def build_nc(s=S, hid=HID, reps=1):
    import concourse.bacc as bacc
    import concourse.mybir as mybir
    from concourse import tile

    f32 = mybir.dt.float32
    f32r = mybir.dt.float32r
    fp8 = mybir.dt.float8e4
    fp16 = mybir.dt.float16
    bf16 = mybir.dt.bfloat16
    Exp = mybir.ActivationFunctionType.Exp
    Sqrt = mybir.ActivationFunctionType.Sqrt
    Copy = mybir.ActivationFunctionType.Copy
    mult = mybir.AluOpType.mult
    subtract = mybir.AluOpType.subtract
    DRow = mybir.MatmulPerfMode.DoubleRow

    def r(ap):
        return ap.bitcast(f32r)

    NCH = s // CH      # s-chunks
    KT = hid // 128    # contraction tiles over HID
    KG = KT // 2       # DoubleRow contraction pairs
    NT = s // 128      # t-tiles
    NPR = NT // 2      # t-tile pairs

    nc = bacc.Bacc("TRN2", target_bir_lowering=False, debug=False,
                   enable_asserts=False, num_devices=NCORES)

    hid_d = nc.dram_tensor("hid8", [hid, s], fp8, kind="ExternalInput").ap()
    hidr_d = nc.dram_tensor("hidr8", [hid, s], fp8, kind="ExternalInput").ap()
    wq_d = nc.dram_tensor("wq8", [hid, 768], fp8, kind="ExternalInput").ap()
    wqr_d = nc.dram_tensor("wqr8", [hid, 768], fp8, kind="ExternalInput").ap()
    wkv_d = nc.dram_tensor("wkv8", [hid, KVR + 2 * DR], fp8, kind="ExternalInput").ap()
    wkvr_d = nc.dram_tensor("wkvr8", [hid, KVR + 2 * DR], fp8, kind="ExternalInput").ap()
    ln_d = nc.dram_tensor("ln_t", [128, 4], f32, kind="ExternalInput").ap()
    kb_d = nc.dram_tensor("kb16", [HL, DN, KVR], fp16, kind="ExternalInput").ap()
    vb_d = nc.dram_tensor("vb16_t", [HL, KVR, DV], fp16, kind="ExternalInput").ap()
    wo_d = nc.dram_tensor("wo8_t", [HL * DV, hid], fp8, kind="ExternalInput").ap()
    wor_d = nc.dram_tensor("wor8_t", [HL * DV, hid], fp8, kind="ExternalInput").ap()
    cs2_d = nc.dram_tensor("cs2", [128, s], bf16, kind="ExternalInput").ap()
    ckr_d = nc.dram_tensor("ckrope", [128, s], bf16, kind="ExternalInput").ap()
    mask_d = nc.dram_tensor("masks", [128, 4, CH], fp8, kind="ExternalInput").ap()
    idm_d = nc.dram_tensor("identm8", [128, 128], fp8, kind="ExternalInput").ap()
    onesq_d = nc.dram_tensor("ones_sq", [128, 128], f32r, kind="ExternalInput").ap()
    on16_d = nc.dram_tensor("ones16", [128, 128], fp16, kind="ExternalInput").ap()
    on8_d = nc.dram_tensor("ones8", [128, 2, 128], fp8, kind="ExternalInput").ap()
    out_d = nc.dram_tensor("out_t", [hid, s], fp16, kind="ExternalOutput").ap()

    with tile.TileContext(nc) as tc, \
         nc.allow_low_precision(reason="fp8/fp16 matmuls; psum accum stays fp32"):
        with tc.tile_pool(name="res", bufs=1) as res, \
             tc.tile_pool(name="psp", bufs=8, space="PSUM") as psp:

            def ps_tile(name):
                return psp.tile([128, CH], f32, tag="ps", name=name)

            # resident tiles
            hid8 = res.tile([128, KT, s], fp8, name="hid8_sb")
            wq8 = res.tile([128, KT, 768], fp8, name="wq8_sb")
            wqr = res.tile([128, KT, 768], fp8, name="wqr_sb")
            wkv8 = res.tile([128, KT, KVR + 2 * DR], fp8, name="wkv8_sb")
            wkvr = res.tile([128, KT, KVR + 2 * DR], fp8, name="wkvr_sb")
            wo8 = res.tile([128, HL, hid], fp8, name="wo8_sb")
            wor = res.tile([128, HL, hid], fp8, name="wor_sb")
            ckvT8 = res.tile([128, 6, s], fp8, name="ckvT8")
            vT16 = res.tile([128, HL, NT, DV], fp16, name="vT16")
            kb16 = res.tile([128, HL, KVR], fp16, name="kb16_sb")
            vb16 = res.tile([128, HL, 4, DV], fp16, name="vb16_sb")
            cs2 = res.tile([128, s], bf16, name="cs2_sb")
            ckr = res.tile([128, s], bf16, name="ckr_sb")
            masks = res.tile([128, 4, CH], fp8, name="masks_sb")
            idm8 = res.tile([128, 128], fp8, name="idm8_sb")
            ln_sb = res.tile([128, 4], f32, name="ln_sb")
            onesq = res.tile([128, 128], f32r, name="onesq_sb")
            on16 = res.tile([128, 128], fp16, name="on16_sb")
            on8 = res.tile([128, 2, 128], fp8, name="on8_sb")
            zb128 = res.tile([128, 1], f32, name="zb128")
            epsb = res.tile([128, 1], f32, name="epsb")
            c_one = res.tile([128, 1], f32, name="c_one")
            cm1 = res.tile([128, 1], f32, name="cm1")
            scr1 = res.tile([1, 4], f32, name="scr1")
            nc.vector.memset(zb128[:], 0.0)
            nc.vector.memset(epsb[:], EPS * WS * WS)
            nc.vector.memset(c_one[:], 1.0)
            nc.vector.memset(cm1[:], -1.0)
            # plane 5 of ckvT8 is an all-zero DoubleRow partner for the rope
            # plane (4): contributes 0 to the score accumulation
            nc.vector.memset(ckvT8[:, 5, :], 0.0)
            # touch every activation function once now so the table loads
            # (1.3us each) hide under the startup DMAs instead of stalling
            # pass 2's first Exp
            nc.scalar.activation(scr1[0:1, 0:1], zb128[0:1, 0:1], Exp)
            nc.scalar.activation(scr1[0:1, 1:2], zb128[0:1, 0:1], Sqrt)
            nc.scalar.activation(scr1[0:1, 2:3], zb128[0:1, 0:1], Copy)

            # DMA queue assignment: sync/scalar carry the pass-1-critical loads
            # (first matmul needs wkv8 + hid8[j0] + hidr[j0]); pass-2-only
            # residents ride the pool queue (Pool engine has slack; SWDGE
            # desc-gen costs ~1us of Pool ENGINE time per copy).
            # first-needed loads split into k-quarters so chunk 0 can start
            # after quarter-transfers on the serial DMA engines
            for kh in range(4):
                ks = slice(kh * (KT // 4), (kh + 1) * (KT // 4))
                rs_ = slice(kh * (hid // 4), (kh + 1) * (hid // 4))
                nc.sync.dma_start(wkv8[:, ks, :],
                                  wkv_d[rs_, :].rearrange("(g p) n -> p g n", p=128))
                nc.scalar.dma_start(hid8[:, ks, 0:CH],
                                    hid_d[rs_, 0:CH].rearrange("(g p) t -> p g t", p=128))
                nc.scalar.dma_start(wkvr[:, ks, :],
                                    wkvr_d[rs_, :].rearrange("(g p) n -> p g n", p=128))
            if NCH > 1:
                nc.scalar.dma_start(hid8[:, :, CH:2 * CH],
                                    hid_d[:, CH:2 * CH].rearrange("(g p) t -> p g t", p=128))
            # post-0 consts before the later hid chunks (those have slack)
            nc.scalar.dma_start(ckr[:], ckr_d)
            nc.scalar.dma_start(ln_sb[:], ln_d)
            nc.scalar.dma_start(onesq[:], onesq_d)
            # vb16 is needed by pass-1 post(0) (v^T precompute); the gpsimd
            # queue is otherwise idle until the gated pass-2 loads
            nc.gpsimd.dma_start(vb16[:], vb_d.rearrange("h (ci p) d -> p h ci d", p=128))
            for j in range(2, NCH):
                sl = slice(j * CH, (j + 1) * CH)
                nc.scalar.dma_start(hid8[:, :, sl],
                                    hid_d[:, sl].rearrange("(g p) t -> p g t", p=128))
            def load_pass2_weights():
                # deferred: these ride the DMA engines behind the pass-1
                # critical streams (hid/hidr per chunk), not ahead of them
                nc.gpsimd.dma_start(cs2[:], cs2_d)
                nc.gpsimd.dma_start(wq8[:], wq_d.rearrange("(g p) n -> p g n", p=128))
                nc.gpsimd.dma_start(wqr[:], wqr_d.rearrange("(g p) n -> p g n", p=128))
                nc.gpsimd.dma_start(kb16[:], kb_d.rearrange("h d c -> d h c"))
                nc.gpsimd.dma_start(masks[:], mask_d)
                nc.gpsimd.dma_start(idm8[:], idm_d)
                nc.gpsimd.dma_start(on16[:], on16_d)
                nc.gpsimd.dma_start(on8[:], on8_d)
                nc.gpsimd.dma_start(wo8[:], wo_d.rearrange("(a p) n -> p a n", p=128))
                nc.gpsimd.dma_start(wor[:], wor_d.rearrange("(a p) n -> p a n", p=128))

            for _rep in range(reps):
              # ---------------- pass 1: latent KV (ckvT8, ckvN8+r, k_pe rot) --
              with tc.tile_pool(name="p1", bufs=1) as p1:
                prev_vt = None
                for j in range(NCH):
                    sl = slice(j * CH, (j + 1) * CH)
                    hidr = p1.tile([128, KT, CH], fp8, tag="hidr", bufs=2, name="hidr")
                    nc.sync.dma_start(hidr[:],
                                      hidr_d[:, sl].rearrange("(g p) t -> p g t", p=128))
                    if j == NCH - 1:
                        # WAW-gate the big pass-2 loads behind the last
                        # critical pass-1 stream: the scheduler orders by data
                        # deps, so dep-free DMAs would otherwise hog the
                        # serial DMA engine ahead of the hid/hidr streams
                        for big in (wq8, wqr, wo8, wor):
                            nc.gpsimd.tensor_copy(big[0:1, 0, 0:1], hidr[0:1, 0, 0:1])
                        nc.gpsimd.tensor_copy(cs2[0:1, 0:1], hidr[0:1, 0, 0:1])
                        load_pass2_weights()
                    cps = [ps_tile(f"cps{ci}") for ci in range(4)]
                    kp_ps = ps_tile("kp_ps")
                    nmb = (KVR + 2 * DR) // 128
                    # mb-major blocks: each output block's accumulation closes
                    # early, so its evac + RMSNorm square overlap later blocks
                    # and the staggered var matmuls never wait on the DVE.
                    # Within a block, terms run (0, 2, 1) so the hidr stream
                    # is not needed until 2/3 through the block.
                    c_sb = [None] * 4
                    sq_box = [None] * 4
                    kp_box = []
                    var_box = []
                    def close_block(mb):
                        if mb < 4:
                            t = p1.tile([128, CH], f32r, tag="c_sb", bufs=5,
                                        name=f"c_sb{mb}")
                            nc.scalar.copy(t[:], cps[mb][:])
                            c_sb[mb] = t
                            sq = p1.tile([128, CH], f32r, tag="sqt", bufs=2, name="sqt")
                            nc.vector.tensor_mul(sq[:], t[:], t[:])
                            sq_box[mb] = sq
                        else:
                            kp = p1.tile([128, CH], f32, tag="kp_sb", bufs=2,
                                         name="kp_sb")
                            nc.scalar.copy(kp[:], kp_ps[:])
                            kp_box.append(kp)

                    if False:
                        # chunk 0 is DMA-startup-bound: term-major gives the
                        # hidr/wkvr streams the longest lead time
                        for ti, term in enumerate((0, 2, 1)):
                            for kg in range(KG):
                                st_ = (ti == 0 and kg == 0)
                                sp_ = (ti == 2 and kg == KG - 1)
                                kk = slice(2 * kg, 2 * kg + 2)
                                w_t, h_t = ((wkv8[:, kk, :], hid8[:, kk, sl]),
                                            (wkv8[:, kk, :], hidr[:, kk, :]),
                                            (wkvr[:, kk, :], hid8[:, kk, sl]))[term]
                                for mb in range(nmb):
                                    mbs = slice(mb * 128, (mb + 1) * 128)
                                    out = cps[mb][:] if mb < 4 else kp_ps[:]
                                    nc.tensor.matmul(out, w_t[:, :, mbs], h_t,
                                                     start=st_, stop=sp_, perf_mode=DRow)
                        for mb in range(nmb):
                            close_block(mb)
                    else:
                        for mb in range(nmb):
                            mbs = slice(mb * 128, (mb + 1) * 128)
                            out = cps[mb][:] if mb < 4 else kp_ps[:]
                            for ti, term in enumerate((0, 2, 1)):
                                for kg in range(KG):
                                    st_ = (ti == 0 and kg == 0)
                                    sp_ = (ti == 2 and kg == KG - 1)
                                    kk = slice(2 * kg, 2 * kg + 2)
                                    w_t, h_t = ((wkv8[:, kk, :], hid8[:, kk, sl]),
                                                (wkv8[:, kk, :], hidr[:, kk, :]),
                                                (wkvr[:, kk, :], hid8[:, kk, sl]))[term]
                                    nc.tensor.matmul(out, w_t[:, :, mbs], h_t,
                                                     start=st_, stop=sp_, perf_mode=DRow)
                            close_block(mb)
                    var_box.append(ps_tile("var_ps"))
                    for ci in range(4):
                        nc.tensor.matmul(var_box[0][:], onesq[:], r(sq_box[ci][:]),
                                         start=(ci == 0), stop=(ci == 3))
                    var_ps = var_box[0]

                    # normalization chain (Act/DVE only; PE moves on to the
                    # next chunk's k-loop meanwhile)
                    sdf = p1.tile([128, CH], f32, tag="sdf", bufs=1, name="sdf")
                    nc.scalar.activation(sdf[:], var_ps[:], Sqrt, bias=epsb[:],
                                         scale=1.0 / KVR)
                    rsq = p1.tile([128, CH], f32r, tag="rsq", bufs=1, name="rsq")
                    nc.vector.reciprocal(rsq[:], sdf[:])
                    ckvT16 = p1.tile([128, 4, CH], fp16, tag="ckvT16", bufs=2,
                                     name="ckvT16")
                    for ci in range(4):
                        nc.vector.scalar_tensor_tensor(ckvT16[:, ci, :], c_sb[ci][:],
                                                       ln_sb[:, ci:ci + 1], rsq[:],
                                                       op0=mult, op1=mult)
                        nc.scalar.copy(ckvT8[:, ci, sl], ckvT16[:, ci, :])
                    # k_pe rope: rows 0:64 = a*cos/WS, 64:128 = b*sin/WS.
                    # Products are written to base-0 slabs (partition shift
                    # rides the psum-input ops), then added partition-aligned.
                    ta_s = p1.tile([32, 2, CH], f32, tag="ta_s", bufs=1, name="ta_s")
                    tb_s = p1.tile([32, 2, CH], f32, tag="tb_s", bufs=1, name="tb_s")
                    kp16 = p1.tile([64, CH], fp16, tag="kp16", bufs=2, name="kp16")
                    for i2 in range(2):
                        nc.vector.tensor_mul(ta_s[:, i2, :],
                                             kp_box[0][32 * i2:32 * i2 + 32, :],
                                             ckr[32 * i2:32 * i2 + 32, sl])
                        nc.vector.tensor_mul(tb_s[:, i2, :],
                                             kp_box[0][64 + 32 * i2:96 + 32 * i2, :],
                                             ckr[64 + 32 * i2:96 + 32 * i2, sl])
                        nc.vector.tensor_add(kp16[32 * i2:32 * i2 + 32, :],
                                             ta_s[:, i2, :], tb_s[:, i2, :])
                    # plane4 = [kp8; kp8], plane5 rows 0:64 = kp residual
                    # (rows 64:128 stay zero): with moving planes
                    # [qp8; qp_res] / [qp8; *] the three products sum to
                    # kp8*qp + kp_r*qp8 ~ kp*qp to second order
                    nc.scalar.activation(ckvT8[0:64, 4, sl], kp16[:], Copy,
                                         scale=1.0)
                    nc.scalar.activation(ckvT8[64:128, 4, sl], kp16[:], Copy,
                                         scale=1.0)
                    nc.vector.scalar_tensor_tensor(ckvT8[0:64, 5, sl], kp16[:],
                                                   c_one[0:64, :],
                                                   ckvT8[0:64, 4, sl],
                                                   op0=mult, op1=subtract)

                    # v^T[t, d] = ckv^T.T @ vb per head (weight-absorbed value
                    # expansion; PV then needs 1 matmul per t-tile).  The PE
                    # matmuls are deferred one chunk so they never wait on the
                    # freshly-written ckvT16.
                    def make_vt(j, ckvT16):
                        def vt():
                            vt_box = [ps_tile(f"vt_ps{h}") for h in range(HL)]
                            for h in range(HL):
                                for q in range(4):
                                    lb = slice(q * 128, (q + 1) * 128)
                                    for ci in range(4):
                                        nc.tensor.matmul(
                                            vt_box[h][:, q * 128:(q + 1) * 128],
                                            ckvT16[:, ci, lb], vb16[:, h, ci, :],
                                            start=(ci == 0), stop=(ci == 3))
                            for h in range(HL):
                                nc.scalar.copy(vT16[:, h, 4 * j:4 * j + 4, :],
                                               vt_box[h][:])
                        return vt

                    vt_j = make_vt(j, ckvT16)
                    if prev_vt is not None:
                        prev_vt()
                    prev_vt = vt_j
                prev_vt()

              # ---------------- pass 2: q proj + attention + o_proj -----------
              with tc.tile_pool(name="p2", bufs=1) as p2:
                prev_oproj = None
                for jo, j in enumerate(range(NCH - 1, -1, -1)):
                    sl = slice(j * CH, (j + 1) * CH)

                    hidr2 = p2.tile([128, KT, CH], fp8, tag="hidr2", bufs=1, name="hidr2")
                    nc.sync.dma_start(hidr2[:],
                                      hidr_d[:, sl].rearrange("(g p) t -> p g t", p=128))
                    qn_ps = [ps_tile(f"qn_ps{h}") for h in range(HL)]
                    qa_ps = [ps_tile(f"qa_ps{p}") for p in range(2)]
                    ql8a = p2.tile([128, 6, HL, CH], fp8, tag="ql8a", bufs=2, name="ql8a")
                    if jo < 2:
                        # plane 5 is a dead DoubleRow partner (zero stationary);
                        # it just has to hold valid fp8 bits, so clear only the
                        # first two ring allocations
                        nc.gpsimd.memset(ql8a[:, 5, :, :], 0.0)
                    qpr16 = p2.tile([64, HL, CH], fp16, tag="qpr16", bufs=1, name="qpr16")
                    qn16 = []
                    # block-major: qa (rope q) first so its DVE rope chain runs
                    # under the qn blocks; terms (0, 2, 1) so hidr2 is needed
                    # only 2/3 into each block; each qn head evacs right after
                    # its block closes
                    def qblock(outs, cols):
                        for ti, term in enumerate((0, 2, 1)):
                            w_t = wq8 if term in (0, 1) else wqr
                            h_t = hidr2[:] if term == 1 else hid8[:, :, sl]
                            for kg in range(KG):
                                st_ = (ti == 0 and kg == 0)
                                sp_ = (ti == 2 and kg == KG - 1)
                                kk = slice(2 * kg, 2 * kg + 2)
                                for out, cb in zip(outs, cols):
                                    nc.tensor.matmul(out[:], w_t[:, kk, cb],
                                                     h_t[:, kk, :], start=st_,
                                                     stop=sp_, perf_mode=DRow)
                    qblock(qa_ps, [slice(512 + p * 128, 512 + (p + 1) * 128)
                                   for p in range(2)])
                    for p in range(2):
                        # q rope: qc = qa*cos; qr = rotate_half(qa)*sin with the
                        # sign flip folded into an stt (cross-partition reads)
                        qc = p2.tile([128, CH], bf16, tag="qc", bufs=1, name="qc")
                        qr = p2.tile([128, CH], bf16, tag="qr", bufs=1, name="qr")
                        for hh in (0, 64):
                            nc.vector.tensor_mul(qc[hh:hh + 64, :], qa_ps[p][hh:hh + 64, :],
                                                 cs2[0:64, sl])
                            nc.vector.scalar_tensor_tensor(qr[hh:hh + 32, :],
                                                           qa_ps[p][hh + 32:hh + 64, :],
                                                           cm1[64:96, :], cs2[64:96, sl],
                                                           op0=mult, op1=mult)
                            nc.vector.tensor_mul(qr[hh + 32:hh + 64, :],
                                                 qa_ps[p][hh:hh + 32, :], cs2[96:128, sl])
                        for i, hh in ((0, 0), (1, 64)):
                            h2 = 2 * p + i
                            nc.vector.tensor_add(qpr16[0:32, h2, :],
                                                 qc[hh:hh + 32, :], qr[hh:hh + 32, :])
                            nc.vector.tensor_add(qpr16[32:64, h2, :],
                                                 qc[hh + 32:hh + 64, :], qr[hh + 32:hh + 64, :])
                    # rope q in fp8: plane4 = [qp8; qp_res], plane5 rows 0:64 =
                    # qp8 again (pairs with the k-side residual)
                    for h in range(HL):
                        nc.scalar.activation(ql8a[0:64, 4, h, :], qpr16[:, h, :],
                                             Copy, scale=1.0)
                        nc.vector.scalar_tensor_tensor(ql8a[64:128, 4, h, :],
                                                       qpr16[:, h, :], c_one[0:64, :],
                                                       ql8a[0:64, 4, h, :],
                                                       op0=mult, op1=subtract)
                        nc.scalar.activation(ql8a[0:64, 5, h, :], qpr16[:, h, :],
                                             Copy, scale=1.0)
                    for h in range(HL):
                        qblock([qn_ps[h]], [slice(h * 128, (h + 1) * 128)])
                        t = p2.tile([128, CH], fp16, tag="qn16", bufs=4, name=f"qn16_{h}")
                        nc.scalar.activation(t[:], qn_ps[h][:], Copy, scale=0.125)
                        qn16.append(t)

                    # previous chunk's o_proj: one ht-quarter is emitted
                    # after each head below, peppering the PE queue so o_proj
                    # matmuls fill the attention pipeline bubbles
                    oproj_quarters = prev_oproj if prev_oproj is not None else []
                    prev_oproj = None

                    vo8a = p2.tile([128, HL, CH], fp8, tag="vo8a", bufs=2, name="vo8a")
                    vor8 = p2.tile([128, HL, CH], fp8, tag="vor8", bufs=2, name="vor8")
                    prev_tail = None
                    for h in range(HL):
                        # q_lat^T[c, s]: plain fp8 matmuls (K=128), evac x 2^-2
                        for ci in range(4):
                            ql_ps = ps_tile("ql_ps")
                            nc.tensor.matmul(ql_ps[:], kb16[:, h, ci * 128:(ci + 1) * 128],
                                             qn16[h][:], start=True, stop=True)
                            if ci % 2 == 0:
                                nc.scalar.activation(ql8a[:, ci, h, :], ql_ps[:], Copy,
                                                     scale=1.0)
                            else:
                                nc.vector.tensor_scalar_mul(ql8a[:, ci, h, :], ql_ps[:], 1.0)

                        # emit the previous head's tail now so its psum-freeing
                        # chain overlaps this head's ql/score matmuls
                        if prev_tail is not None:
                            prev_tail()
                            prev_tail = None

                        # t-pair order: diagonal pairs first, then history pairs
                        prs = [(2 * j, (0, 0), True), (2 * j + 1, (256, 384), True)] + \
                              [(m, (0, 0), False) for m in range(0, 2 * j)]

                        def do_pair(m, sts, diag):
                            e8p = None if diag else p2.tile([128, 2, CH], fp8,
                                                            tag="e8p", bufs=5, name="e8p")
                            exs = []
                            for par in range(2):
                                st = sts[par]
                                t_i = 2 * m + par
                                tb = slice(t_i * 128, (t_i + 1) * 128)
                                sc_ps = ps_tile("sc_ps")
                                nc.tensor.matmul(sc_ps[:, st:], ckvT8[:, 4:6, tb],
                                                 ql8a[:, 4:6, h, st:],
                                                 start=True, stop=False, perf_mode=DRow)
                                nc.tensor.matmul(sc_ps[:, st:], ckvT8[:, 0:2, tb],
                                                 ql8a[:, 0:2, h, st:],
                                                 start=False, stop=False, perf_mode=DRow)
                                if diag:
                                    # mask add as a tiny fp16 identity-matmul on
                                    # the PE, folded into the score accumulation
                                    kd = t_i - 4 * j
                                    ma, mb2 = ((0, 128), (0, 256),
                                               (256, 384), (384, 512))[kd]
                                    nc.tensor.matmul(sc_ps[:, ma:mb2], idm8[:],
                                                     masks[:, kd, ma:mb2],
                                                     start=False, stop=False)
                                nc.tensor.matmul(sc_ps[:, st:], ckvT8[:, 2:4, tb],
                                                 ql8a[:, 2:4, h, st:],
                                                 start=False, stop=True, perf_mode=DRow)
                                ex16 = p2.tile([128, CH], fp16, tag="ex16", bufs=6, name="ex16")
                                nc.scalar.activation(ex16[:, st:], sc_ps[:, st:], Exp,
                                                     bias=zb128[:], scale=SCALE / WS)
                                if e8p is not None:
                                    nc.vector.tensor_copy(e8p[:, par, :], ex16[:])
                                exs.append(ex16)
                            return tuple(exs) + (e8p,)

                        vo_box = []
                        rs_box = []

                        def pv(idx, m, sts, ex_a, ex_b, e8p):
                            first, last = (idx == 0), (idx == len(prs) - 1)
                            for par, ext in ((0, ex_a), (1, ex_b)):
                                st = sts[par]
                                t_i = 2 * m + par
                                nc.tensor.matmul(vo_box[0][:, st:],
                                                 vT16[:, h, t_i, :], ext[:, st:],
                                                 start=(first and par == 0),
                                                 stop=(last and par == 1))
                                if e8p is None:
                                    nc.tensor.matmul(rs_box[0][:, st:], on16[:], ext[:, st:],
                                                     start=(first and par == 0),
                                                     stop=(last and par == 1))
                            if e8p is not None:
                                nc.tensor.matmul(rs_box[0][:, :], on8[:], e8p[:],
                                                 start=first, stop=last, perf_mode=DRow)

                        pend = []
                        for idx, (m, sts, diag) in enumerate(prs):
                            pair_t = do_pair(m, sts, diag)
                            if idx == 0:
                                vo_box.append(ps_tile("vo_ps"))
                                rs_box.append(ps_tile("rs_ps"))
                            pend.append((idx, m, sts) + pair_t)
                            if len(pend) > 4:
                                pv(*pend.pop(0))
                        for pd in pend:
                            pv(*pd)

                        def make_tail(h, vo_ps, rs_ps):
                            def tail():
                                # softmax denominator: full-row reciprocal on DVE
                                rbc = p2.tile([128, CH], f32r, tag="rbc", bufs=1, name="rbc")
                                nc.vector.reciprocal(rbc[:], rs_ps[:])
                                # normalize v-out, fp8 + residual split
                                tmp16 = p2.tile([128, CH], fp16, tag="tmp16", bufs=2, name="tmp16")
                                nc.vector.scalar_tensor_tensor(tmp16[:], vo_ps[:],
                                                               c_one[:], rbc[:],
                                                               op0=mult, op1=mult)
                                nc.gpsimd.tensor_copy(vo8a[:, h, :], tmp16[:])
                                nc.gpsimd.tensor_tensor(vor8[:, h, :], tmp16[:],
                                                        vo8a[:, h, :], op=subtract)
                            return tail

                        prev_tail = make_tail(h, vo_box[0], rs_box[0])
                        if h < len(oproj_quarters):
                            oproj_quarters[h]()
                    prev_tail()

                    # o_proj partial (3-term fp8x2): out^T = sum_h wo^T.T @ v_out^T
                    def make_oproj(sl, vo8a, vor8):
                        def oproj(hts):
                            for ht in hts:
                                htb = slice(ht * 128, (ht + 1) * 128)
                                oo_ps = ps_tile("oo_ps")
                                for g2 in range(2):
                                    hh2 = slice(2 * g2, 2 * g2 + 2)
                                    nc.tensor.matmul(oo_ps[:], wo8[:, hh2, htb], vo8a[:, hh2, :],
                                                     start=(g2 == 0), stop=False, perf_mode=DRow)
                                    nc.tensor.matmul(oo_ps[:], wo8[:, hh2, htb], vor8[:, hh2, :],
                                                     start=False, stop=False, perf_mode=DRow)
                                    nc.tensor.matmul(oo_ps[:], wor[:, hh2, htb], vo8a[:, hh2, :],
                                                     start=False, stop=(g2 == 1), perf_mode=DRow)
                                oo_sb = p2.tile([128, CH], fp16, tag="oo_sb", bufs=3, name="oo_sb")
                                if ht % 2 == 0:
                                    nc.scalar.activation(oo_sb[:], oo_ps[:], Copy,
                                                         scale=1.0 / (WS * WS))
                                else:
                                    nc.vector.tensor_scalar_mul(oo_sb[:], oo_ps[:],
                                                                1.0 / (WS * WS))
                                if ht % 3 == 0:
                                    nc.sync.dma_start(out_d[htb, sl], oo_sb[:])
                                elif ht % 3 == 1:
                                    nc.scalar.dma_start(out_d[htb, sl], oo_sb[:])
                                else:
                                    nc.gpsimd.dma_start(out_d[htb, sl], oo_sb[:])
                        return oproj

                    _op = make_oproj(sl, vo8a, vor8)
                    prev_oproj = [
                        (lambda q=q, f=_op: f(range(4 * q, 4 * q + 4)))
                        for q in range(4)]
                for q in prev_oproj:
                    q()

    nc.compile()
    return nc


# ---------------------------------------------------------------------------
# host-side input prep / output assembly
# ---------------------------------------------------------------------------
_PERM = np.concatenate([np.arange(0, DR, 2), np.arange(1, DR, 2)])


def _rope_tables(pos, s):
    inv_freq = 1.0 / (THETA ** (np.arange(0, DR, 2, dtype=np.float64) / DR))
    t = pos.astype(np.float64)
    freqs = t[:, None] * inv_freq
    emb = np.concatenate([freqs, freqs], axis=-1)          # [s, DR]
    cosT = np.cos(emb).T.astype(np.float32)                # [DR, s]
    sinT = np.sin(emb).T.astype(np.float32)
    return cosT, sinT


def _masks():
    t = np.arange(128)[:, None]
    c = np.arange(CH)[None, :]
    m = np.zeros((128, 4, CH), np.float32)
    for kd in range(4):
        m[:, kd, :] = np.where(c >= 128 * kd + t, 0.0, -30000.0).astype(np.float32)
    return m


def _fp8_split(x):
    a = x.astype(FP8)
    r = (x - a.astype(np.float32)).astype(FP8)
    return a, r


def prep_core_inputs(inputs, core, s=S, hid=HID):
    b, g = core // 4, core % 4
    heads = slice(HL * g, HL * (g + 1))
    hs = np.asarray(inputs["hidden_states"], np.float32)[b, :s, :hid]
    m = {}
    m["hid8"], m["hidr8"] = _fp8_split(np.ascontiguousarray(hs.T))

    wq = np.asarray(inputs["q_nope_weight"], np.float32).reshape(H, DN, HID)[heads, :, :hid]
    wq_t = wq.transpose(2, 0, 1).reshape(hid, HL * DN)
    wqp = np.asarray(inputs["q_pe_weight"], np.float32).reshape(H, DR, HID)[heads, :, :hid]
    a = wqp[:, _PERM, :]                                   # [4, 64, hid]
    A = a.reshape(2, 128, hid)
    wqpe_t = np.concatenate([A[0], A[1]], axis=0).T
    m["wq8"], m["wqr8"] = _fp8_split(np.concatenate([wq_t, wqpe_t], axis=1) * WS)

    wkv = np.asarray(inputs["kv_a_weight"], np.float32)[:, :hid]
    kpe_a = wkv[KVR:][_PERM]
    kpe_b = np.concatenate([-kpe_a[32:], kpe_a[:32]], axis=0)
    wkv_t = np.ascontiguousarray(
        np.concatenate([wkv[:KVR], kpe_a, kpe_b], axis=0).T * WS)
    m["wkv8"], m["wkvr8"] = _fp8_split(wkv_t)

    m["ln_t"] = np.ascontiguousarray(
        np.asarray(inputs["kv_a_ln_weight"], np.float32).reshape(4, 128).T)
    m["kb16"] = (np.asarray(inputs["k_b_weight"], np.float32)[heads] * (WS / 4)).astype(np.float16)
    m["vb16_t"] = np.ascontiguousarray(
        np.asarray(inputs["v_b_weight"], np.float32)[heads].transpose(0, 2, 1) * WS
    ).astype(np.float16)
    wo_t = np.ascontiguousarray(
        np.asarray(inputs["o_weight"], np.float32)[:hid, HL * DV * g:HL * DV * (g + 1)].T * WS)
    m["wo8_t"], m["wor8_t"] = _fp8_split(wo_t)

    pos = np.asarray(inputs["position_ids"]).reshape(-1)[:s]
    cosT, sinT = _rope_tables(pos, s)                      # [64, s] each
    m["cs2"] = np.ascontiguousarray(np.vstack([cosT, sinT])).astype(BF16)
    m["ckrope"] = np.ascontiguousarray(
        np.vstack([cosT, sinT]) * (1.0 / WS)).astype(BF16)
    m["masks"] = (_masks() / 128.0).astype(FP8)
    m["identm8"] = (np.eye(128, dtype=np.float32) * 128.0).astype(FP8)
    m["ones_sq"] = np.ones((128, 128), np.float32)
    m["ones16"] = np.ones((128, 128), np.float16)
    m["ones8"] = np.ones((128, 2, 128), np.float32).astype(FP8)
    return m


_NC_CACHE = {}


def _get_nc():
    if "nc" not in _NC_CACHE:
        _NC_CACHE["nc"] = build_nc()
    return _NC_CACHE["nc"]


def kernel(**inputs):
    from concourse import bass_utils

    nc = _get_nc()
    in_maps = [prep_core_inputs(inputs, c) for c in range(NCORES)]
    res = bass_utils.run_bass_kernel_spmd(nc, in_maps, core_ids=list(range(NCORES)))
    out = np.empty((B, S, HID), np.float32)
    for b in range(B):
        acc = np.array(res.results[4 * b]["out_t"], np.float32)
        for g in range(1, 4):
            acc += res.results[4 * b + g]["out_t"]
        out[b] = acc.T
    return out



# revision 62
# speedup vs baseline: 1.0288x; 1.0061x over previous
"""DeepseekV2 MLA attention (weight-absorbed, MQA-style latent) on 8 TRN2 NeuronCores.

Sharding: data-parallel over batch (B=2) x tensor-parallel over heads (4 heads/core).
Each core computes, for its batch element and its 4 heads, the partial o_proj
output out_t = [HID, S] (transposed layout); the host sums the 4 partials per
batch element and transposes back.

Mixed-precision design.  The only fast PE mode is fp8e4m3 with DoubleRow
(2x128-deep contraction per instruction at 0.5 cycles/row), so:
  * The large projections (q, ckv, o_proj) run as 3-term fp8 DoubleRow
    products with host-side fp8 residual splits (x = x8 + r8):
    x8*y8 + x8*ry + rx*y8, keeping ~0.1-0.5% error at 1.5x the plain-fp8
    cost (4x cheaper than fp32r would be).
  * The score matmul's latent part is fp8 DoubleRow against fp8-quantized
    ckvT/q_lat (that noise enters through the softmax exponent and is
    damped); the rope part, q_lat (K=128) and causal-mask adds run on the
    PE in fp16/fp8 at 1.0 cycles/row, where fp8 would gain nothing.
  * The PV matmul, its rs row-sum (diagonal pairs) and v_b run in fp16 so
    the exp() output feeds the PV matmul directly with no per-tile
    requantization chain; history-pair row-sums use an fp8 DoubleRow
    ones-matmul over a pool-copied fp8 ex (error ~rs_err/sqrt(history)).

Scale ledger (log2 of stored/true): hid8 0 | wq/wkv/kb/vb/wo +5 | qn16 +2 |
ql8 +5 | qpr16 +5 | kper16 0 (k-rope tables pre-divided by 32) | ckv latents
0 (RMSNorm cancels +5; eps pre-scaled 2^10) | scores +5 (exp scale SCALE/32;
masks are fp8 -234 applied via a x128 fp8 identity matmul) | ex 0 |
ol16 -6 unnormalized | vo8a/vor8 +5 normalized | oo_ps +10 -> out x 2^-10.

Softmax is max-free (score magnitudes are small; verified on host).
"""
import sys

for _p in ("/opt/trn_rl_repo", "/root/.axon_site/_ro/trn_rl_repo"):
    if _p not in sys.path:
        sys.path.insert(0, _p)

import numpy as np
import ml_dtypes

B, S, HID = 2, 2048, 2048
H, DN, DR, KVR, DV = 16, 128, 64, 512, 128
THETA, EPS = 10000.0, 1e-6
SCALE = float((DN + DR) ** -0.5)
NCORES, HL = 8, 4  # 2 (batch) x 4 (head groups of 4)
CH = 512           # s-chunk width (= psum bank width in fp32)
WS = 32.0          # host-side weight pre-scale (2^5)

FP8 = ml_dtypes.float8_e4m3
BF16 = ml_dtypes.bfloat16


def build_nc(s=S, hid=HID, reps=1):
    import concourse.bacc as bacc
    import concourse.mybir as mybir
    from concourse import tile

    f32 = mybir.dt.float32
    f32r = mybir.dt.float32r
    fp8 = mybir.dt.float8e4
    fp16 = mybir.dt.float16
    bf16 = mybir.dt.bfloat16
    Exp = mybir.ActivationFunctionType.Exp
    Sqrt = mybir.ActivationFunctionType.Sqrt
    Copy = mybir.ActivationFunctionType.Copy
    mult = mybir.AluOpType.mult
    subtract = mybir.AluOpType.subtract
    DRow = mybir.MatmulPerfMode.DoubleRow

    def r(ap):
        return ap.bitcast(f32r)

    NCH = s // CH      # s-chunks
    KT = hid // 128    # contraction tiles over HID
    KG = KT // 2       # DoubleRow contraction pairs
    NT = s // 128      # t-tiles
    NPR = NT // 2      # t-tile pairs

    nc = bacc.Bacc("TRN2", target_bir_lowering=False, debug=False,
                   enable_asserts=False, num_devices=NCORES)

    hid_d = nc.dram_tensor("hid8", [hid, s], fp8, kind="ExternalInput").ap()
    hidr_d = nc.dram_tensor("hidr8", [hid, s], fp8, kind="ExternalInput").ap()
    wq_d = nc.dram_tensor("wq8", [hid, 768], fp8, kind="ExternalInput").ap()
    wqr_d = nc.dram_tensor("wqr8", [hid, 768], fp8, kind="ExternalInput").ap()
    wkv_d = nc.dram_tensor("wkv8", [hid, KVR + 2 * DR], fp8, kind="ExternalInput").ap()
    wkvr_d = nc.dram_tensor("wkvr8", [hid, KVR + 2 * DR], fp8, kind="ExternalInput").ap()
    ln_d = nc.dram_tensor("ln_t", [128, 4], f32, kind="ExternalInput").ap()
    kb_d = nc.dram_tensor("kb16", [HL, DN, KVR], fp16, kind="ExternalInput").ap()
    vb_d = nc.dram_tensor("vb16_t", [HL, KVR, DV], fp16, kind="ExternalInput").ap()
    wo_d = nc.dram_tensor("wo8_t", [HL * DV, hid], fp8, kind="ExternalInput").ap()
    wor_d = nc.dram_tensor("wor8_t", [HL * DV, hid], fp8, kind="ExternalInput").ap()
    cs2_d = nc.dram_tensor("cs2", [128, s], bf16, kind="ExternalInput").ap()
    ckr_d = nc.dram_tensor("ckrope", [128, s], bf16, kind="ExternalInput").ap()
    mask_d = nc.dram_tensor("masks", [128, 4, CH], fp8, kind="ExternalInput").ap()
    idm_d = nc.dram_tensor("identm8", [128, 128], fp8, kind="ExternalInput").ap()
    onesq_d = nc.dram_tensor("ones_sq", [128, 128], f32r, kind="ExternalInput").ap()
    on16_d = nc.dram_tensor("ones16", [128, 128], fp16, kind="ExternalInput").ap()
    on8_d = nc.dram_tensor("ones8", [128, 2, 128], fp8, kind="ExternalInput").ap()
    out_d = nc.dram_tensor("out_t", [hid, s], fp16, kind="ExternalOutput").ap()

    with tile.TileContext(nc) as tc, \
         nc.allow_low_precision(reason="fp8/fp16 matmuls; psum accum stays fp32"):
        with tc.tile_pool(name="res", bufs=1) as res, \
             tc.tile_pool(name="psp", bufs=8, space="PSUM") as psp:

            def ps_tile(name):
                return psp.tile([128, CH], f32, tag="ps", name=name)

            # resident tiles
            hid8 = res.tile([128, KT, s], fp8, name="hid8_sb")
            wq8 = res.tile([128, KT, 768], fp8, name="wq8_sb")
            wqr = res.tile([128, KT, 768], fp8, name="wqr_sb")
            wkv8 = res.tile([128, KT, KVR + 2 * DR], fp8, name="wkv8_sb")
            wkvr = res.tile([128, KT, KVR + 2 * DR], fp8, name="wkvr_sb")
            wo8 = res.tile([128, HL, hid], fp8, name="wo8_sb")
            wor = res.tile([128, HL, hid], fp8, name="wor_sb")
            ckvT8 = res.tile([128, 6, s], fp8, name="ckvT8")
            vT16 = res.tile([128, HL, NT, DV], fp16, name="vT16")
            kb16 = res.tile([128, HL, KVR], fp16, name="kb16_sb")
            vb16 = res.tile([128, HL, 4, DV], fp16, name="vb16_sb")
            cs2 = res.tile([128, s], bf16, name="cs2_sb")
            ckr = res.tile([128, s], bf16, name="ckr_sb")
            masks = res.tile([128, 4, CH], fp8, name="masks_sb")
            idm8 = res.tile([128, 128], fp8, name="idm8_sb")
            ln_sb = res.tile([128, 4], f32, name="ln_sb")
            onesq = res.tile([128, 128], f32r, name="onesq_sb")
            on16 = res.tile([128, 128], fp16, name="on16_sb")
            on8 = res.tile([128, 2, 128], fp8, name="on8_sb")
            zb128 = res.tile([128, 1], f32, name="zb128")
            epsb = res.tile([128, 1], f32, name="epsb")
            c_one = res.tile([128, 1], f32, name="c_one")
            cm1 = res.tile([128, 1], f32, name="cm1")
            scr1 = res.tile([1, 4], f32, name="scr1")
            nc.vector.memset(zb128[:], 0.0)
            nc.vector.memset(epsb[:], EPS * WS * WS)
            nc.vector.memset(c_one[:], 1.0)
            nc.vector.memset(cm1[:], -1.0)
            # plane 5 of ckvT8 is an all-zero DoubleRow partner for the rope
            # plane (4): contributes 0 to the score accumulation
            nc.vector.memset(ckvT8[:, 5, :], 0.0)
            # touch every activation function once now so the table loads
            # (1.3us each) hide under the startup DMAs instead of stalling
            # pass 2's first Exp
            nc.scalar.activation(scr1[0:1, 0:1], zb128[0:1, 0:1], Exp)
            nc.scalar.activation(scr1[0:1, 1:2], zb128[0:1, 0:1], Sqrt)
            nc.scalar.activation(scr1[0:1, 2:3], zb128[0:1, 0:1], Copy)

            # DMA queue assignment: sync/scalar carry the pass-1-critical loads
            # (first matmul needs wkv8 + hid8[j0] + hidr[j0]); pass-2-only
            # residents ride the pool queue (Pool engine has slack; SWDGE
            # desc-gen costs ~1us of Pool ENGINE time per copy).
            # first-needed loads split into k-quarters so chunk 0 can start
            # after quarter-transfers on the serial DMA engines
            for kh in range(4):
                ks = slice(kh * (KT // 4), (kh + 1) * (KT // 4))
                rs_ = slice(kh * (hid // 4), (kh + 1) * (hid // 4))
                nc.sync.dma_start(wkv8[:, ks, :],
                                  wkv_d[rs_, :].rearrange("(g p) n -> p g n", p=128))
                nc.scalar.dma_start(hid8[:, ks, 0:CH],
                                    hid_d[rs_, 0:CH].rearrange("(g p) t -> p g t", p=128))
                nc.scalar.dma_start(wkvr[:, ks, :],
                                    wkvr_d[rs_, :].rearrange("(g p) n -> p g n", p=128))
            if NCH > 1:
                nc.scalar.dma_start(hid8[:, :, CH:2 * CH],
                                    hid_d[:, CH:2 * CH].rearrange("(g p) t -> p g t", p=128))
            # post-0 consts before the later hid chunks (those have slack)
            nc.scalar.dma_start(ckr[:], ckr_d)
            nc.scalar.dma_start(ln_sb[:], ln_d)
            nc.scalar.dma_start(onesq[:], onesq_d)
            # vb16 is needed by pass-1 post(0) (v^T precompute); the gpsimd
            # queue is otherwise idle until the gated pass-2 loads
            nc.gpsimd.dma_start(vb16[:], vb_d.rearrange("h (ci p) d -> p h ci d", p=128))
            for j in range(2, NCH):
                sl = slice(j * CH, (j + 1) * CH)
                nc.scalar.dma_start(hid8[:, :, sl],
                                    hid_d[:, sl].rearrange("(g p) t -> p g t", p=128))
            def load_pass2_weights():
                # deferred: these ride the DMA engines behind the pass-1
                # critical streams (hid/hidr per chunk), not ahead of them
                nc.gpsimd.dma_start(cs2[:], cs2_d)
                nc.gpsimd.dma_start(wq8[:], wq_d.rearrange("(g p) n -> p g n", p=128))
                nc.gpsimd.dma_start(wqr[:], wqr_d.rearrange("(g p) n -> p g n", p=128))
                nc.gpsimd.dma_start(kb16[:], kb_d.rearrange("h d c -> d h c"))
                nc.gpsimd.dma_start(masks[:], mask_d)
                nc.gpsimd.dma_start(idm8[:], idm_d)
                nc.gpsimd.dma_start(on16[:], on16_d)
                nc.gpsimd.dma_start(on8[:], on8_d)
                nc.gpsimd.dma_start(wo8[:], wo_d.rearrange("(a p) n -> p a n", p=128))
                nc.gpsimd.dma_start(wor[:], wor_d.rearrange("(a p) n -> p a n", p=128))

            for _rep in range(reps):
              # ---------------- pass 1: latent KV (ckvT8, ckvN8+r, k_pe rot) --
              with tc.tile_pool(name="p1", bufs=1) as p1:
                prev_vt = None
                for j in range(NCH):
                    sl = slice(j * CH, (j + 1) * CH)
                    hidr = p1.tile([128, KT, CH], fp8, tag="hidr", bufs=2, name="hidr")
                    nc.sync.dma_start(hidr[:],
                                      hidr_d[:, sl].rearrange("(g p) t -> p g t", p=128))
                    if j == NCH - 1:
                        # WAW-gate the big pass-2 loads behind the last
                        # critical pass-1 stream: the scheduler orders by data
                        # deps, so dep-free DMAs would otherwise hog the
                        # serial DMA engine ahead of the hid/hidr streams
                        for big in (wq8, wqr, wo8, wor):
                            nc.gpsimd.tensor_copy(big[0:1, 0, 0:1], hidr[0:1, 0, 0:1])
                        nc.gpsimd.tensor_copy(cs2[0:1, 0:1], hidr[0:1, 0, 0:1])
                        load_pass2_weights()
                    cps = [ps_tile(f"cps{ci}") for ci in range(4)]
                    kp_ps = ps_tile("kp_ps")
                    nmb = (KVR + 2 * DR) // 128
                    # mb-major blocks: each output block's accumulation closes
                    # early, so its evac + RMSNorm square overlap later blocks
                    # and the staggered var matmuls never wait on the DVE.
                    # Within a block, terms run (0, 2, 1) so the hidr stream
                    # is not needed until 2/3 through the block.
                    c_sb = [None] * 4
                    sq_box = [None] * 4
                    kp_box = []
                    var_box = []
                    def close_block(mb):
                        if mb < 4:
                            t = p1.tile([128, CH], f32r, tag="c_sb", bufs=5,
                                        name=f"c_sb{mb}")
                            nc.scalar.copy(t[:], cps[mb][:])
                            c_sb[mb] = t
                            sq = p1.tile([128, CH], f32r, tag="sqt", bufs=2, name="sqt")
                            nc.vector.tensor_mul(sq[:], t[:], t[:])
                            sq_box[mb] = sq
                        else:
                            kp = p1.tile([128, CH], f32, tag="kp_sb", bufs=2,
                                         name="kp_sb")
                            nc.scalar.copy(kp[:], kp_ps[:])
                            kp_box.append(kp)

                    if False:
                        # chunk 0 is DMA-startup-bound: term-major gives the
                        # hidr/wkvr streams the longest lead time
                        for ti, term in enumerate((0, 2, 1)):
                            for kg in range(KG):
                                st_ = (ti == 0 and kg == 0)
                                sp_ = (ti == 2 and kg == KG - 1)
                                kk = slice(2 * kg, 2 * kg + 2)
                                w_t, h_t = ((wkv8[:, kk, :], hid8[:, kk, sl]),
                                            (wkv8[:, kk, :], hidr[:, kk, :]),
                                            (wkvr[:, kk, :], hid8[:, kk, sl]))[term]
                                for mb in range(nmb):
                                    mbs = slice(mb * 128, (mb + 1) * 128)
                                    out = cps[mb][:] if mb < 4 else kp_ps[:]
                                    nc.tensor.matmul(out, w_t[:, :, mbs], h_t,
                                                     start=st_, stop=sp_, perf_mode=DRow)
                        for mb in range(nmb):
                            close_block(mb)
                    else:
                        for mb in range(nmb):
                            mbs = slice(mb * 128, (mb + 1) * 128)
                            out = cps[mb][:] if mb < 4 else kp_ps[:]
                            for ti, term in enumerate((0, 2, 1)):
                                for kg in range(KG):
                                    st_ = (ti == 0 and kg == 0)
                                    sp_ = (ti == 2 and kg == KG - 1)
                                    kk = slice(2 * kg, 2 * kg + 2)
                                    w_t, h_t = ((wkv8[:, kk, :], hid8[:, kk, sl]),
                                                (wkv8[:, kk, :], hidr[:, kk, :]),
                                                (wkvr[:, kk, :], hid8[:, kk, sl]))[term]
                                    nc.tensor.matmul(out, w_t[:, :, mbs], h_t,
                                                     start=st_, stop=sp_, perf_mode=DRow)
                            close_block(mb)
                    var_box.append(ps_tile("var_ps"))
                    for ci in range(4):
                        nc.tensor.matmul(var_box[0][:], onesq[:], r(sq_box[ci][:]),
                                         start=(ci == 0), stop=(ci == 3))
                    var_ps = var_box[0]

                    # normalization chain (Act/DVE only; PE moves on to the
                    # next chunk's k-loop meanwhile)
                    sdf = p1.tile([128, CH], f32, tag="sdf", bufs=1, name="sdf")
                    nc.scalar.activation(sdf[:], var_ps[:], Sqrt, bias=epsb[:],
                                         scale=1.0 / KVR)
                    rsq = p1.tile([128, CH], f32r, tag="rsq", bufs=1, name="rsq")
                    nc.vector.reciprocal(rsq[:], sdf[:])
                    ckvT16 = p1.tile([128, 4, CH], fp16, tag="ckvT16", bufs=2,
                                     name="ckvT16")
                    for ci in range(4):
                        nc.vector.scalar_tensor_tensor(ckvT16[:, ci, :], c_sb[ci][:],
                                                       ln_sb[:, ci:ci + 1], rsq[:],
                                                       op0=mult, op1=mult)
                        nc.scalar.copy(ckvT8[:, ci, sl], ckvT16[:, ci, :])
                    # k_pe rope: rows 0:64 = a*cos/WS, 64:128 = b*sin/WS.
                    # Products are written to base-0 slabs (partition shift
                    # rides the psum-input ops), then added partition-aligned.
                    ta_s = p1.tile([32, 2, CH], f32, tag="ta_s", bufs=1, name="ta_s")
                    tb_s = p1.tile([32, 2, CH], f32, tag="tb_s", bufs=1, name="tb_s")
                    kp16 = p1.tile([64, CH], fp16, tag="kp16", bufs=2, name="kp16")
                    for i2 in range(2):
                        nc.vector.tensor_mul(ta_s[:, i2, :],
                                             kp_box[0][32 * i2:32 * i2 + 32, :],
                                             ckr[32 * i2:32 * i2 + 32, sl])
                        nc.vector.tensor_mul(tb_s[:, i2, :],
                                             kp_box[0][64 + 32 * i2:96 + 32 * i2, :],
                                             ckr[64 + 32 * i2:96 + 32 * i2, sl])
                        nc.vector.tensor_add(kp16[32 * i2:32 * i2 + 32, :],
                                             ta_s[:, i2, :], tb_s[:, i2, :])
                    # plane4 = [kp8; kp8], plane5 rows 0:64 = kp residual
                    # (rows 64:128 stay zero): with moving planes
                    # [qp8; qp_res] / [qp8; *] the three products sum to
                    # kp8*qp + kp_r*qp8 ~ kp*qp to second order
                    nc.scalar.activation(ckvT8[0:64, 4, sl], kp16[:], Copy,
                                         scale=1.0)
                    nc.scalar.activation(ckvT8[64:128, 4, sl], kp16[:], Copy,
                                         scale=1.0)
                    nc.vector.scalar_tensor_tensor(ckvT8[0:64, 5, sl], kp16[:],
                                                   c_one[0:64, :],
                                                   ckvT8[0:64, 4, sl],
                                                   op0=mult, op1=subtract)

                    # v^T[t, d] = ckv^T.T @ vb per head (weight-absorbed value
                    # expansion; PV then needs 1 matmul per t-tile).  The PE
                    # matmuls are deferred one chunk so they never wait on the
                    # freshly-written ckvT16.
                    def make_vt(j, ckvT16):
                        def vt():
                            vt_box = [ps_tile(f"vt_ps{h}") for h in range(HL)]
                            for h in range(HL):
                                for q in range(4):
                                    lb = slice(q * 128, (q + 1) * 128)
                                    for ci in range(4):
                                        nc.tensor.matmul(
                                            vt_box[h][:, q * 128:(q + 1) * 128],
                                            ckvT16[:, ci, lb], vb16[:, h, ci, :],
                                            start=(ci == 0), stop=(ci == 3))
                            for h in range(HL):
                                nc.scalar.copy(vT16[:, h, 4 * j:4 * j + 4, :],
                                               vt_box[h][:])
                        return vt

                    vt_j = make_vt(j, ckvT16)
                    if prev_vt is not None:
                        prev_vt()
                    prev_vt = vt_j
                prev_vt()

              # ---------------- pass 2: q proj + attention + o_proj -----------
              with tc.tile_pool(name="p2", bufs=1) as p2:
                prev_oproj = None
                for jo, j in enumerate(range(NCH - 1, -1, -1)):
                    sl = slice(j * CH, (j + 1) * CH)

                    hidr2 = p2.tile([128, KT, CH], fp8, tag="hidr2", bufs=1, name="hidr2")
                    nc.sync.dma_start(hidr2[:],
                                      hidr_d[:, sl].rearrange("(g p) t -> p g t", p=128))
                    qn_ps = [ps_tile(f"qn_ps{h}") for h in range(HL)]
                    qa_ps = [ps_tile(f"qa_ps{p}") for p in range(2)]
                    ql8a = p2.tile([128, 6, HL, CH], fp8, tag="ql8a", bufs=2, name="ql8a")
                    if jo < 2:
                        # plane 5 is a dead DoubleRow partner (zero stationary);
                        # it just has to hold valid fp8 bits, so clear only the
                        # first two ring allocations
                        nc.gpsimd.memset(ql8a[:, 5, :, :], 0.0)
                    qpr16 = p2.tile([64, HL, CH], fp16, tag="qpr16", bufs=1, name="qpr16")
                    qn16 = []
                    # block-major: qa (rope q) first so its DVE rope chain runs
                    # under the qn blocks; terms (0, 2, 1) so hidr2 is needed
                    # only 2/3 into each block; each qn head evacs right after
                    # its block closes
                    def qblock(outs, cols):
                        for ti, term in enumerate((0, 2, 1)):
                            w_t = wq8 if term in (0, 1) else wqr
                            h_t = hidr2[:] if term == 1 else hid8[:, :, sl]
                            for kg in range(KG):
                                st_ = (ti == 0 and kg == 0)
                                sp_ = (ti == 2 and kg == KG - 1)
                                kk = slice(2 * kg, 2 * kg + 2)
                                for out, cb in zip(outs, cols):
                                    nc.tensor.matmul(out[:], w_t[:, kk, cb],
                                                     h_t[:, kk, :], start=st_,
                                                     stop=sp_, perf_mode=DRow)
                    qblock(qa_ps, [slice(512 + p * 128, 512 + (p + 1) * 128)
                                   for p in range(2)])
                    for p in range(2):
                        # q rope: qc = qa*cos; qr = rotate_half(qa)*sin with the
                        # sign flip folded into an stt (cross-partition reads)
                        qc = p2.tile([128, CH], bf16, tag="qc", bufs=1, name="qc")
                        qr = p2.tile([128, CH], bf16, tag="qr", bufs=1, name="qr")
                        for hh in (0, 64):
                            nc.vector.tensor_mul(qc[hh:hh + 64, :], qa_ps[p][hh:hh + 64, :],
                                                 cs2[0:64, sl])
                            nc.vector.scalar_tensor_tensor(qr[hh:hh + 32, :],
                                                           qa_ps[p][hh + 32:hh + 64, :],
                                                           cm1[64:96, :], cs2[64:96, sl],
                                                           op0=mult, op1=mult)
                            nc.vector.tensor_mul(qr[hh + 32:hh + 64, :],
                                                 qa_ps[p][hh:hh + 32, :], cs2[96:128, sl])
                        for i, hh in ((0, 0), (1, 64)):
                            h2 = 2 * p + i
                            nc.vector.tensor_add(qpr16[0:32, h2, :],
                                                 qc[hh:hh + 32, :], qr[hh:hh + 32, :])
                            nc.vector.tensor_add(qpr16[32:64, h2, :],
                                                 qc[hh + 32:hh + 64, :], qr[hh + 32:hh + 64, :])
                    # rope q in fp8: plane4 = [qp8; qp_res], plane5 rows 0:64 =
                    # qp8 again (pairs with the k-side residual)
                    for h in range(HL):
                        nc.scalar.activation(ql8a[0:64, 4, h, :], qpr16[:, h, :],
                                             Copy, scale=1.0)
                        nc.vector.scalar_tensor_tensor(ql8a[64:128, 4, h, :],
                                                       qpr16[:, h, :], c_one[0:64, :],
                                                       ql8a[0:64, 4, h, :],
                                                       op0=mult, op1=subtract)
                        nc.scalar.activation(ql8a[0:64, 5, h, :], qpr16[:, h, :],
                                             Copy, scale=1.0)
                    for h in range(HL):
                        qblock([qn_ps[h]], [slice(h * 128, (h + 1) * 128)])
                        t = p2.tile([128, CH], fp16, tag="qn16", bufs=4, name=f"qn16_{h}")
                        nc.scalar.activation(t[:], qn_ps[h][:], Copy, scale=0.125)
                        qn16.append(t)

                    # previous chunk's o_proj: one ht-quarter is emitted
                    # after each head below, peppering the PE queue so o_proj
                    # matmuls fill the attention pipeline bubbles
                    oproj_quarters = prev_oproj if prev_oproj is not None else []
                    prev_oproj = None

                    vo8a = p2.tile([128, HL, CH], fp8, tag="vo8a", bufs=2, name="vo8a")
                    vor8 = p2.tile([128, HL, CH], fp8, tag="vor8", bufs=2, name="vor8")
                    prev_tail = None
                    for h in range(HL):
                        # q_lat^T[c, s]: plain fp8 matmuls (K=128), evac x 2^-2
                        for ci in range(4):
                            ql_ps = ps_tile("ql_ps")
                            nc.tensor.matmul(ql_ps[:], kb16[:, h, ci * 128:(ci + 1) * 128],
                                             qn16[h][:], start=True, stop=True)
                            if ci % 2 == 0:
                                nc.scalar.activation(ql8a[:, ci, h, :], ql_ps[:], Copy,
                                                     scale=1.0)
                            else:
                                nc.vector.tensor_scalar_mul(ql8a[:, ci, h, :], ql_ps[:], 1.0)

                        # emit the previous head's tail now so its psum-freeing
                        # chain overlaps this head's ql/score matmuls
                        if prev_tail is not None:
                            prev_tail()
                            prev_tail = None

                        # t-pair order: diagonal pairs first, then history pairs
                        prs = [(2 * j, (0, 0), True), (2 * j + 1, (256, 384), True)] + \
                              [(m, (0, 0), False) for m in range(0, 2 * j)]

                        def do_pair(m, sts, diag):
                            e8p = None if diag else p2.tile([128, 2, CH], fp8,
                                                            tag="e8p", bufs=5, name="e8p")
                            exs = []
                            for par in range(2):
                                st = sts[par]
                                t_i = 2 * m + par
                                tb = slice(t_i * 128, (t_i + 1) * 128)
                                sc_ps = ps_tile("sc_ps")
                                nc.tensor.matmul(sc_ps[:, st:], ckvT8[:, 4:6, tb],
                                                 ql8a[:, 4:6, h, st:],
                                                 start=True, stop=False, perf_mode=DRow)
                                if diag:
                                    # mask add as a tiny fp16 identity-matmul on
                                    # the PE, folded into the score accumulation
                                    # (inputs are resident, so it can issue
                                    # while the ql8a latent evacs land)
                                    kd = t_i - 4 * j
                                    ma, mb2 = ((0, 128), (0, 256),
                                               (256, 384), (384, 512))[kd]
                                    nc.tensor.matmul(sc_ps[:, ma:mb2], idm8[:],
                                                     masks[:, kd, ma:mb2],
                                                     start=False, stop=False)
                                nc.tensor.matmul(sc_ps[:, st:], ckvT8[:, 0:2, tb],
                                                 ql8a[:, 0:2, h, st:],
                                                 start=False, stop=False, perf_mode=DRow)
                                nc.tensor.matmul(sc_ps[:, st:], ckvT8[:, 2:4, tb],
                                                 ql8a[:, 2:4, h, st:],
                                                 start=False, stop=True, perf_mode=DRow)
                                ex16 = p2.tile([128, CH], fp16, tag="ex16", bufs=6, name="ex16")
                                nc.scalar.activation(ex16[:, st:], sc_ps[:, st:], Exp,
                                                     bias=zb128[:], scale=SCALE / WS)
                                if e8p is not None:
                                    nc.vector.tensor_copy(e8p[:, par, :], ex16[:])
                                exs.append(ex16)
                            return tuple(exs) + (e8p,)

                        vo_box = []
                        rs_box = []

                        def pv(idx, m, sts, ex_a, ex_b, e8p):
                            first, last = (idx == 0), (idx == len(prs) - 1)
                            for par, ext in ((0, ex_a), (1, ex_b)):
                                st = sts[par]
                                t_i = 2 * m + par
                                nc.tensor.matmul(vo_box[0][:, st:],
                                                 vT16[:, h, t_i, :], ext[:, st:],
                                                 start=(first and par == 0),
                                                 stop=(last and par == 1))
                                if e8p is None:
                                    nc.tensor.matmul(rs_box[0][:, st:], on16[:], ext[:, st:],
                                                     start=(first and par == 0),
                                                     stop=(last and par == 1))
                            if e8p is not None:
                                nc.tensor.matmul(rs_box[0][:, :], on8[:], e8p[:],
                                                 start=first, stop=last, perf_mode=DRow)

                        pend = []
                        for idx, (m, sts, diag) in enumerate(prs):
                            pair_t = do_pair(m, sts, diag)
                            if idx == 0:
                                vo_box.append(ps_tile("vo_ps"))
                                rs_box.append(ps_tile("rs_ps"))
                            pend.append((idx, m, sts) + pair_t)
                            if len(pend) > 4:
                                pv(*pend.pop(0))
                        for pd in pend:
                            pv(*pd)

                        def make_tail(h, vo_ps, rs_ps):
                            def tail():
                                # softmax denominator: full-row reciprocal on DVE
                                rbc = p2.tile([128, CH], f32r, tag="rbc", bufs=1, name="rbc")
                                nc.vector.reciprocal(rbc[:], rs_ps[:])
                                # normalize v-out, fp8 + residual split
                                tmp16 = p2.tile([128, CH], fp16, tag="tmp16", bufs=2, name="tmp16")
                                nc.vector.scalar_tensor_tensor(tmp16[:], vo_ps[:],
                                                               c_one[:], rbc[:],
                                                               op0=mult, op1=mult)
                                nc.gpsimd.tensor_copy(vo8a[:, h, :], tmp16[:])
                                nc.vector.scalar_tensor_tensor(vor8[:, h, :], tmp16[:],
                                                               c_one[:], vo8a[:, h, :],
                                                               op0=mult, op1=subtract)
                            return tail

                        prev_tail = make_tail(h, vo_box[0], rs_box[0])
                        if h < len(oproj_quarters):
                            oproj_quarters[h]()
                    prev_tail()

                    # o_proj partial (3-term fp8x2): out^T = sum_h wo^T.T @ v_out^T
                    def make_oproj(sl, vo8a, vor8):
                        def oproj(hts):
                            for ht in hts:
                                htb = slice(ht * 128, (ht + 1) * 128)
                                oo_ps = ps_tile("oo_ps")
                                for g2 in range(2):
                                    hh2 = slice(2 * g2, 2 * g2 + 2)
                                    nc.tensor.matmul(oo_ps[:], wo8[:, hh2, htb], vo8a[:, hh2, :],
                                                     start=(g2 == 0), stop=False, perf_mode=DRow)
                                    nc.tensor.matmul(oo_ps[:], wo8[:, hh2, htb], vor8[:, hh2, :],
                                                     start=False, stop=False, perf_mode=DRow)
                                    nc.tensor.matmul(oo_ps[:], wor[:, hh2, htb], vo8a[:, hh2, :],
                                                     start=False, stop=(g2 == 1), perf_mode=DRow)
                                oo_sb = p2.tile([128, CH], fp16, tag="oo_sb", bufs=3, name="oo_sb")
                                if ht % 2 == 0:
                                    nc.scalar.activation(oo_sb[:], oo_ps[:], Copy,
                                                         scale=1.0 / (WS * WS))
                                else:
                                    nc.vector.tensor_scalar_mul(oo_sb[:], oo_ps[:],
                                                                1.0 / (WS * WS))
                                if ht % 3 == 0:
                                    nc.sync.dma_start(out_d[htb, sl], oo_sb[:])
                                elif ht % 3 == 1:
                                    nc.scalar.dma_start(out_d[htb, sl], oo_sb[:])
                                else:
                                    nc.gpsimd.dma_start(out_d[htb, sl], oo_sb[:])
                        return oproj

                    _op = make_oproj(sl, vo8a, vor8)
                    prev_oproj = [
                        (lambda q=q, f=_op: f(range(4 * q, 4 * q + 4)))
                        for q in range(4)]
                for q in prev_oproj:
                    q()

    nc.compile()
    return nc


# ---------------------------------------------------------------------------
# host-side input prep / output assembly
# ---------------------------------------------------------------------------
_PERM = np.concatenate([np.arange(0, DR, 2), np.arange(1, DR, 2)])


def _rope_tables(pos, s):
    inv_freq = 1.0 / (THETA ** (np.arange(0, DR, 2, dtype=np.float64) / DR))
    t = pos.astype(np.float64)
    freqs = t[:, None] * inv_freq
    emb = np.concatenate([freqs, freqs], axis=-1)          # [s, DR]
    cosT = np.cos(emb).T.astype(np.float32)                # [DR, s]
    sinT = np.sin(emb).T.astype(np.float32)
    return cosT, sinT


def _masks():
    t = np.arange(128)[:, None]
    c = np.arange(CH)[None, :]
    m = np.zeros((128, 4, CH), np.float32)
    for kd in range(4):
        m[:, kd, :] = np.where(c >= 128 * kd + t, 0.0, -30000.0).astype(np.float32)
    return m


def _fp8_split(x):
    a = x.astype(FP8)
    r = (x - a.astype(np.float32)).astype(FP8)
    return a, r


def prep_core_inputs(inputs, core, s=S, hid=HID):
    b, g = core // 4, core % 4
    heads = slice(HL * g, HL * (g + 1))
    hs = np.asarray(inputs["hidden_states"], np.float32)[b, :s, :hid]
    m = {}
    m["hid8"], m["hidr8"] = _fp8_split(np.ascontiguousarray(hs.T))

    wq = np.asarray(inputs["q_nope_weight"], np.float32).reshape(H, DN, HID)[heads, :, :hid]
    wq_t = wq.transpose(2, 0, 1).reshape(hid, HL * DN)
    wqp = np.asarray(inputs["q_pe_weight"], np.float32).reshape(H, DR, HID)[heads, :, :hid]
    a = wqp[:, _PERM, :]                                   # [4, 64, hid]
    A = a.reshape(2, 128, hid)
    wqpe_t = np.concatenate([A[0], A[1]], axis=0).T
    m["wq8"], m["wqr8"] = _fp8_split(np.concatenate([wq_t, wqpe_t], axis=1) * WS)

    wkv = np.asarray(inputs["kv_a_weight"], np.float32)[:, :hid]
    kpe_a = wkv[KVR:][_PERM]
    kpe_b = np.concatenate([-kpe_a[32:], kpe_a[:32]], axis=0)
    wkv_t = np.ascontiguousarray(
        np.concatenate([wkv[:KVR], kpe_a, kpe_b], axis=0).T * WS)
    m["wkv8"], m["wkvr8"] = _fp8_split(wkv_t)

    m["ln_t"] = np.ascontiguousarray(
        np.asarray(inputs["kv_a_ln_weight"], np.float32).reshape(4, 128).T)
    m["kb16"] = (np.asarray(inputs["k_b_weight"], np.float32)[heads] * (WS / 4)).astype(np.float16)
    m["vb16_t"] = np.ascontiguousarray(
        np.asarray(inputs["v_b_weight"], np.float32)[heads].transpose(0, 2, 1) * WS
    ).astype(np.float16)
    wo_t = np.ascontiguousarray(
        np.asarray(inputs["o_weight"], np.float32)[:hid, HL * DV * g:HL * DV * (g + 1)].T * WS)
    m["wo8_t"], m["wor8_t"] = _fp8_split(wo_t)

    pos = np.asarray(inputs["position_ids"]).reshape(-1)[:s]
    cosT, sinT = _rope_tables(pos, s)                      # [64, s] each
    m["cs2"] = np.ascontiguousarray(np.vstack([cosT, sinT])).astype(BF16)
    m["ckrope"] = np.ascontiguousarray(
        np.vstack([cosT, sinT]) * (1.0 / WS)).astype(BF16)
    m["masks"] = (_masks() / 128.0).astype(FP8)
    m["identm8"] = (np.eye(128, dtype=np.float32) * 128.0).astype(FP8)
    m["ones_sq"] = np.ones((128, 128), np.float32)
    m["ones16"] = np.ones((128, 128), np.float16)
    m["ones8"] = np.ones((128, 2, 128), np.float32).astype(FP8)
    return m


_NC_CACHE = {}


def _get_nc():
    if "nc" not in _NC_CACHE:
        _NC_CACHE["nc"] = build_nc()
    return _NC_CACHE["nc"]


def kernel(**inputs):
    from concourse import bass_utils

    nc = _get_nc()
    in_maps = [prep_core_inputs(inputs, c) for c in range(NCORES)]
    res = bass_utils.run_bass_kernel_spmd(nc, in_maps, core_ids=list(range(NCORES)))
    out = np.empty((B, S, HID), np.float32)
    for b in range(B):
        acc = np.array(res.results[4 * b]["out_t"], np.float32)
        for g in range(1, 4):
            acc += res.results[4 * b + g]["out_t"]
        out[b] = acc.T
    return out



# revision 63
# speedup vs baseline: 1.0312x; 1.0023x over previous
"""DeepseekV2 MLA attention (weight-absorbed, MQA-style latent) on 8 TRN2 NeuronCores.

Sharding: data-parallel over batch (B=2) x tensor-parallel over heads (4 heads/core).
Each core computes, for its batch element and its 4 heads, the partial o_proj
output out_t = [HID, S] (transposed layout); the host sums the 4 partials per
batch element and transposes back.

Mixed-precision design.  The only fast PE mode is fp8e4m3 with DoubleRow
(2x128-deep contraction per instruction at 0.5 cycles/row), so:
  * The large projections (q, ckv, o_proj) run as 3-term fp8 DoubleRow
    products with host-side fp8 residual splits (x = x8 + r8):
    x8*y8 + x8*ry + rx*y8, keeping ~0.1-0.5% error at 1.5x the plain-fp8
    cost (4x cheaper than fp32r would be).
  * The score matmul's latent part is fp8 DoubleRow against fp8-quantized
    ckvT/q_lat (that noise enters through the softmax exponent and is
    damped); the rope part, q_lat (K=128) and causal-mask adds run on the
    PE in fp16/fp8 at 1.0 cycles/row, where fp8 would gain nothing.
  * The PV matmul, its rs row-sum (diagonal pairs) and v_b run in fp16 so
    the exp() output feeds the PV matmul directly with no per-tile
    requantization chain; history-pair row-sums use an fp8 DoubleRow
    ones-matmul over a pool-copied fp8 ex (error ~rs_err/sqrt(history)).

Scale ledger (log2 of stored/true): hid8 0 | wq/wkv/kb/vb/wo +5 | qn16 +2 |
ql8 +5 | qpr16 +5 | kper16 0 (k-rope tables pre-divided by 32) | ckv latents
0 (RMSNorm cancels +5; eps pre-scaled 2^10) | scores +5 (exp scale SCALE/32;
masks are fp8 -234 applied via a x128 fp8 identity matmul) | ex 0 |
ol16 -6 unnormalized | vo8a/vor8 +5 normalized | oo_ps +10 -> out x 2^-10.

Softmax is max-free (score magnitudes are small; verified on host).
"""
import sys

for _p in ("/opt/trn_rl_repo", "/root/.axon_site/_ro/trn_rl_repo"):
    if _p not in sys.path:
        sys.path.insert(0, _p)

import numpy as np
import ml_dtypes

B, S, HID = 2, 2048, 2048
H, DN, DR, KVR, DV = 16, 128, 64, 512, 128
THETA, EPS = 10000.0, 1e-6
SCALE = float((DN + DR) ** -0.5)
NCORES, HL = 8, 4  # 2 (batch) x 4 (head groups of 4)
CH = 512           # s-chunk width (= psum bank width in fp32)
WS = 32.0          # host-side weight pre-scale (2^5)

FP8 = ml_dtypes.float8_e4m3
BF16 = ml_dtypes.bfloat16


def build_nc(s=S, hid=HID, reps=1):
    import concourse.bacc as bacc
    import concourse.mybir as mybir
    from concourse import tile

    f32 = mybir.dt.float32
    f32r = mybir.dt.float32r
    fp8 = mybir.dt.float8e4
    fp16 = mybir.dt.float16
    bf16 = mybir.dt.bfloat16
    Exp = mybir.ActivationFunctionType.Exp
    Sqrt = mybir.ActivationFunctionType.Sqrt
    Copy = mybir.ActivationFunctionType.Copy
    mult = mybir.AluOpType.mult
    subtract = mybir.AluOpType.subtract
    DRow = mybir.MatmulPerfMode.DoubleRow

    def r(ap):
        return ap.bitcast(f32r)

    NCH = s // CH      # s-chunks
    KT = hid // 128    # contraction tiles over HID
    KG = KT // 2       # DoubleRow contraction pairs
    NT = s // 128      # t-tiles
    NPR = NT // 2      # t-tile pairs

    nc = bacc.Bacc("TRN2", target_bir_lowering=False, debug=False,
                   enable_asserts=False, num_devices=NCORES)

    hid_d = nc.dram_tensor("hid8", [hid, s], fp8, kind="ExternalInput").ap()
    hidr_d = nc.dram_tensor("hidr8", [hid, s], fp8, kind="ExternalInput").ap()
    wq_d = nc.dram_tensor("wq8", [hid, 768], fp8, kind="ExternalInput").ap()
    wqr_d = nc.dram_tensor("wqr8", [hid, 768], fp8, kind="ExternalInput").ap()
    wkv_d = nc.dram_tensor("wkv8", [hid, KVR + 2 * DR], fp8, kind="ExternalInput").ap()
    wkvr_d = nc.dram_tensor("wkvr8", [hid, KVR + 2 * DR], fp8, kind="ExternalInput").ap()
    ln_d = nc.dram_tensor("ln_t", [128, 4], f32, kind="ExternalInput").ap()
    kb_d = nc.dram_tensor("kb16", [HL, DN, KVR], fp16, kind="ExternalInput").ap()
    vb_d = nc.dram_tensor("vb16_t", [HL, KVR, DV], fp16, kind="ExternalInput").ap()
    wo_d = nc.dram_tensor("wo8_t", [HL * DV, hid], fp8, kind="ExternalInput").ap()
    wor_d = nc.dram_tensor("wor8_t", [HL * DV, hid], fp8, kind="ExternalInput").ap()
    cs2_d = nc.dram_tensor("cs2", [128, s], bf16, kind="ExternalInput").ap()
    ckr_d = nc.dram_tensor("ckrope", [128, s], bf16, kind="ExternalInput").ap()
    mask_d = nc.dram_tensor("masks", [128, 4, CH], fp8, kind="ExternalInput").ap()
    idm_d = nc.dram_tensor("identm8", [128, 128], fp8, kind="ExternalInput").ap()
    onesq_d = nc.dram_tensor("ones_sq", [128, 128], f32r, kind="ExternalInput").ap()
    on16_d = nc.dram_tensor("ones16", [128, 128], fp16, kind="ExternalInput").ap()
    on8_d = nc.dram_tensor("ones8", [128, 2, 128], fp8, kind="ExternalInput").ap()
    out_d = nc.dram_tensor("out_t", [hid, s], fp16, kind="ExternalOutput").ap()

    with tile.TileContext(nc) as tc, \
         nc.allow_low_precision(reason="fp8/fp16 matmuls; psum accum stays fp32"):
        with tc.tile_pool(name="res", bufs=1) as res, \
             tc.tile_pool(name="psp", bufs=8, space="PSUM") as psp:

            def ps_tile(name):
                return psp.tile([128, CH], f32, tag="ps", name=name)

            # resident tiles
            hid8 = res.tile([128, KT, s], fp8, name="hid8_sb")
            wq8 = res.tile([128, KT, 768], fp8, name="wq8_sb")
            wqr = res.tile([128, KT, 768], fp8, name="wqr_sb")
            wkv8 = res.tile([128, KT, KVR + 2 * DR], fp8, name="wkv8_sb")
            wkvr = res.tile([128, KT, KVR + 2 * DR], fp8, name="wkvr_sb")
            wo8 = res.tile([128, HL, hid], fp8, name="wo8_sb")
            wor = res.tile([128, HL, hid], fp8, name="wor_sb")
            ckvT8 = res.tile([128, 6, s], fp8, name="ckvT8")
            vT16 = res.tile([128, HL, NT, DV], fp16, name="vT16")
            kb16 = res.tile([128, HL, KVR], fp16, name="kb16_sb")
            vb16 = res.tile([128, HL, 4, DV], fp16, name="vb16_sb")
            cs2 = res.tile([128, s], bf16, name="cs2_sb")
            ckr = res.tile([128, s], bf16, name="ckr_sb")
            masks = res.tile([128, 4, CH], fp8, name="masks_sb")
            idm8 = res.tile([128, 128], fp8, name="idm8_sb")
            ln_sb = res.tile([128, 4], f32, name="ln_sb")
            onesq = res.tile([128, 128], f32r, name="onesq_sb")
            on16 = res.tile([128, 128], fp16, name="on16_sb")
            on8 = res.tile([128, 2, 128], fp8, name="on8_sb")
            zb128 = res.tile([128, 1], f32, name="zb128")
            epsb = res.tile([128, 1], f32, name="epsb")
            c_one = res.tile([128, 1], f32, name="c_one")
            cm1 = res.tile([128, 1], f32, name="cm1")
            scr1 = res.tile([1, 4], f32, name="scr1")
            nc.vector.memset(zb128[:], 0.0)
            nc.vector.memset(epsb[:], EPS * WS * WS)
            nc.vector.memset(c_one[:], 1.0)
            nc.vector.memset(cm1[:], -1.0)
            # plane 5 of ckvT8 is an all-zero DoubleRow partner for the rope
            # plane (4): contributes 0 to the score accumulation
            nc.vector.memset(ckvT8[:, 5, :], 0.0)
            # touch every activation function once now so the table loads
            # (1.3us each) hide under the startup DMAs instead of stalling
            # pass 2's first Exp
            nc.scalar.activation(scr1[0:1, 0:1], zb128[0:1, 0:1], Exp)
            nc.scalar.activation(scr1[0:1, 1:2], zb128[0:1, 0:1], Sqrt)
            nc.scalar.activation(scr1[0:1, 2:3], zb128[0:1, 0:1], Copy)

            # DMA queue assignment: sync/scalar carry the pass-1-critical loads
            # (first matmul needs wkv8 + hid8[j0] + hidr[j0]); pass-2-only
            # residents ride the pool queue (Pool engine has slack; SWDGE
            # desc-gen costs ~1us of Pool ENGINE time per copy).
            # first-needed loads split into k-quarters so chunk 0 can start
            # after quarter-transfers on the serial DMA engines
            for kh in range(4):
                ks = slice(kh * (KT // 4), (kh + 1) * (KT // 4))
                rs_ = slice(kh * (hid // 4), (kh + 1) * (hid // 4))
                nc.sync.dma_start(wkv8[:, ks, :],
                                  wkv_d[rs_, :].rearrange("(g p) n -> p g n", p=128))
                nc.scalar.dma_start(hid8[:, ks, 0:CH],
                                    hid_d[rs_, 0:CH].rearrange("(g p) t -> p g t", p=128))
                nc.scalar.dma_start(wkvr[:, ks, :],
                                    wkvr_d[rs_, :].rearrange("(g p) n -> p g n", p=128))
            if NCH > 1:
                nc.scalar.dma_start(hid8[:, :, CH:2 * CH],
                                    hid_d[:, CH:2 * CH].rearrange("(g p) t -> p g t", p=128))
            # post-0 consts before the later hid chunks (those have slack)
            nc.scalar.dma_start(ckr[:], ckr_d)
            nc.scalar.dma_start(ln_sb[:], ln_d)
            nc.scalar.dma_start(onesq[:], onesq_d)
            # vb16 is needed by pass-1 post(0) (v^T precompute); the gpsimd
            # queue is otherwise idle until the gated pass-2 loads
            nc.gpsimd.dma_start(vb16[:], vb_d.rearrange("h (ci p) d -> p h ci d", p=128))
            for j in range(2, NCH):
                sl = slice(j * CH, (j + 1) * CH)
                nc.scalar.dma_start(hid8[:, :, sl],
                                    hid_d[:, sl].rearrange("(g p) t -> p g t", p=128))
            def load_pass2_weights():
                # deferred: these ride the DMA engines behind the pass-1
                # critical streams (hid/hidr per chunk), not ahead of them
                nc.gpsimd.dma_start(cs2[:], cs2_d)
                nc.gpsimd.dma_start(wq8[:], wq_d.rearrange("(g p) n -> p g n", p=128))
                nc.gpsimd.dma_start(wqr[:], wqr_d.rearrange("(g p) n -> p g n", p=128))
                nc.gpsimd.dma_start(kb16[:], kb_d.rearrange("h d c -> d h c"))
                nc.gpsimd.dma_start(masks[:], mask_d)
                nc.gpsimd.dma_start(idm8[:], idm_d)
                nc.gpsimd.dma_start(on16[:], on16_d)
                nc.gpsimd.dma_start(on8[:], on8_d)
                nc.gpsimd.dma_start(wo8[:], wo_d.rearrange("(a p) n -> p a n", p=128))
                nc.gpsimd.dma_start(wor[:], wor_d.rearrange("(a p) n -> p a n", p=128))

            for _rep in range(reps):
              # ---------------- pass 1: latent KV (ckvT8, ckvN8+r, k_pe rot) --
              with tc.tile_pool(name="p1", bufs=1) as p1:
                prev_vt = None
                for j in range(NCH):
                    sl = slice(j * CH, (j + 1) * CH)
                    hidr = p1.tile([128, KT, CH], fp8, tag="hidr", bufs=2, name="hidr")
                    nc.sync.dma_start(hidr[:],
                                      hidr_d[:, sl].rearrange("(g p) t -> p g t", p=128))
                    if j == NCH - 1:
                        # WAW-gate the big pass-2 loads behind the last
                        # critical pass-1 stream: the scheduler orders by data
                        # deps, so dep-free DMAs would otherwise hog the
                        # serial DMA engine ahead of the hid/hidr streams
                        for big in (wq8, wqr, wo8, wor):
                            nc.gpsimd.tensor_copy(big[0:1, 0, 0:1], hidr[0:1, 0, 0:1])
                        nc.gpsimd.tensor_copy(cs2[0:1, 0:1], hidr[0:1, 0, 0:1])
                        load_pass2_weights()
                    cps = [ps_tile(f"cps{ci}") for ci in range(4)]
                    kp_ps = ps_tile("kp_ps")
                    nmb = (KVR + 2 * DR) // 128
                    # mb-major blocks: each output block's accumulation closes
                    # early, so its evac + RMSNorm square overlap later blocks
                    # and the staggered var matmuls never wait on the DVE.
                    # Within a block, terms run (0, 2, 1) so the hidr stream
                    # is not needed until 2/3 through the block.
                    c_sb = [None] * 4
                    sq_box = [None] * 4
                    kp_box = []
                    var_box = []
                    def close_block(mb):
                        if mb < 4:
                            t = p1.tile([128, CH], f32r, tag="c_sb", bufs=5,
                                        name=f"c_sb{mb}")
                            nc.scalar.copy(t[:], cps[mb][:])
                            c_sb[mb] = t
                            sq = p1.tile([128, CH], f32r, tag="sqt", bufs=2, name="sqt")
                            nc.vector.tensor_mul(sq[:], t[:], t[:])
                            sq_box[mb] = sq
                        else:
                            kp = p1.tile([128, CH], f32, tag="kp_sb", bufs=2,
                                         name="kp_sb")
                            nc.scalar.copy(kp[:], kp_ps[:])
                            kp_box.append(kp)

                    if False:
                        # chunk 0 is DMA-startup-bound: term-major gives the
                        # hidr/wkvr streams the longest lead time
                        for ti, term in enumerate((0, 2, 1)):
                            for kg in range(KG):
                                st_ = (ti == 0 and kg == 0)
                                sp_ = (ti == 2 and kg == KG - 1)
                                kk = slice(2 * kg, 2 * kg + 2)
                                w_t, h_t = ((wkv8[:, kk, :], hid8[:, kk, sl]),
                                            (wkv8[:, kk, :], hidr[:, kk, :]),
                                            (wkvr[:, kk, :], hid8[:, kk, sl]))[term]
                                for mb in range(nmb):
                                    mbs = slice(mb * 128, (mb + 1) * 128)
                                    out = cps[mb][:] if mb < 4 else kp_ps[:]
                                    nc.tensor.matmul(out, w_t[:, :, mbs], h_t,
                                                     start=st_, stop=sp_, perf_mode=DRow)
                        for mb in range(nmb):
                            close_block(mb)
                    else:
                        for mb in range(nmb):
                            mbs = slice(mb * 128, (mb + 1) * 128)
                            out = cps[mb][:] if mb < 4 else kp_ps[:]
                            for ti, term in enumerate((0, 2, 1)):
                                for kg in range(KG):
                                    st_ = (ti == 0 and kg == 0)
                                    sp_ = (ti == 2 and kg == KG - 1)
                                    kk = slice(2 * kg, 2 * kg + 2)
                                    w_t, h_t = ((wkv8[:, kk, :], hid8[:, kk, sl]),
                                                (wkv8[:, kk, :], hidr[:, kk, :]),
                                                (wkvr[:, kk, :], hid8[:, kk, sl]))[term]
                                    nc.tensor.matmul(out, w_t[:, :, mbs], h_t,
                                                     start=st_, stop=sp_, perf_mode=DRow)
                            close_block(mb)
                    var_box.append(ps_tile("var_ps"))
                    for ci in range(4):
                        nc.tensor.matmul(var_box[0][:], onesq[:], r(sq_box[ci][:]),
                                         start=(ci == 0), stop=(ci == 3))
                    var_ps = var_box[0]

                    # normalization chain (Act/DVE only; PE moves on to the
                    # next chunk's k-loop meanwhile)
                    sdf = p1.tile([128, CH], f32, tag="sdf", bufs=1, name="sdf")
                    nc.scalar.activation(sdf[:], var_ps[:], Sqrt, bias=epsb[:],
                                         scale=1.0 / KVR)
                    rsq = p1.tile([128, CH], f32r, tag="rsq", bufs=1, name="rsq")
                    nc.vector.reciprocal(rsq[:], sdf[:])
                    ckvT16 = p1.tile([128, 4, CH], fp16, tag="ckvT16", bufs=2,
                                     name="ckvT16")
                    for ci in range(4):
                        nc.vector.scalar_tensor_tensor(ckvT16[:, ci, :], c_sb[ci][:],
                                                       ln_sb[:, ci:ci + 1], rsq[:],
                                                       op0=mult, op1=mult)
                        nc.scalar.copy(ckvT8[:, ci, sl], ckvT16[:, ci, :])
                    # k_pe rope: rows 0:64 = a*cos/WS, 64:128 = b*sin/WS.
                    # Products are written to base-0 slabs (partition shift
                    # rides the psum-input ops), then added partition-aligned.
                    ta_s = p1.tile([32, 2, CH], f32, tag="ta_s", bufs=1, name="ta_s")
                    tb_s = p1.tile([32, 2, CH], f32, tag="tb_s", bufs=1, name="tb_s")
                    kp16 = p1.tile([64, CH], fp16, tag="kp16", bufs=2, name="kp16")
                    for i2 in range(2):
                        nc.vector.tensor_mul(ta_s[:, i2, :],
                                             kp_box[0][32 * i2:32 * i2 + 32, :],
                                             ckr[32 * i2:32 * i2 + 32, sl])
                        nc.vector.tensor_mul(tb_s[:, i2, :],
                                             kp_box[0][64 + 32 * i2:96 + 32 * i2, :],
                                             ckr[64 + 32 * i2:96 + 32 * i2, sl])
                        nc.vector.tensor_add(kp16[32 * i2:32 * i2 + 32, :],
                                             ta_s[:, i2, :], tb_s[:, i2, :])
                    # plane4 = [kp8; kp8], plane5 rows 0:64 = kp residual
                    # (rows 64:128 stay zero): with moving planes
                    # [qp8; qp_res] / [qp8; *] the three products sum to
                    # kp8*qp + kp_r*qp8 ~ kp*qp to second order
                    nc.scalar.activation(ckvT8[0:64, 4, sl], kp16[:], Copy,
                                         scale=1.0)
                    nc.scalar.activation(ckvT8[64:128, 4, sl], kp16[:], Copy,
                                         scale=1.0)
                    nc.vector.scalar_tensor_tensor(ckvT8[0:64, 5, sl], kp16[:],
                                                   c_one[0:64, :],
                                                   ckvT8[0:64, 4, sl],
                                                   op0=mult, op1=subtract)

                    # v^T[t, d] = ckv^T.T @ vb per head (weight-absorbed value
                    # expansion; PV then needs 1 matmul per t-tile).  The PE
                    # matmuls are deferred one chunk so they never wait on the
                    # freshly-written ckvT16.
                    def make_vt(j, ckvT16):
                        def vt():
                            vt_box = [ps_tile(f"vt_ps{h}") for h in range(HL)]
                            for h in range(HL):
                                for q in range(4):
                                    lb = slice(q * 128, (q + 1) * 128)
                                    for ci in range(4):
                                        nc.tensor.matmul(
                                            vt_box[h][:, q * 128:(q + 1) * 128],
                                            ckvT16[:, ci, lb], vb16[:, h, ci, :],
                                            start=(ci == 0), stop=(ci == 3))
                            for h in range(HL):
                                nc.scalar.copy(vT16[:, h, 4 * j:4 * j + 4, :],
                                               vt_box[h][:])
                        return vt

                    vt_j = make_vt(j, ckvT16)
                    if prev_vt is not None:
                        prev_vt()
                    prev_vt = vt_j
                prev_vt()

              # ---------------- pass 2: q proj + attention + o_proj -----------
              with tc.tile_pool(name="p2", bufs=1) as p2:
                prev_oproj = None
                for jo, j in enumerate(range(NCH - 1, -1, -1)):
                    sl = slice(j * CH, (j + 1) * CH)

                    hidr2 = p2.tile([128, KT, CH], fp8, tag="hidr2", bufs=1, name="hidr2")
                    nc.sync.dma_start(hidr2[:],
                                      hidr_d[:, sl].rearrange("(g p) t -> p g t", p=128))
                    qn_ps = [ps_tile(f"qn_ps{h}") for h in range(HL)]
                    qa_ps = [ps_tile(f"qa_ps{p}") for p in range(2)]
                    ql8a = p2.tile([128, 6, HL, CH], fp8, tag="ql8a", bufs=2, name="ql8a")
                    if jo < 2:
                        # plane 5 is a dead DoubleRow partner (zero stationary);
                        # it just has to hold valid fp8 bits, so clear only the
                        # first two ring allocations
                        nc.gpsimd.memset(ql8a[:, 5, :, :], 0.0)
                    qpr16 = p2.tile([64, HL, CH], fp16, tag="qpr16", bufs=1, name="qpr16")
                    qn16 = []
                    # block-major: qa (rope q) first so its DVE rope chain runs
                    # under the qn blocks; terms (0, 2, 1) so hidr2 is needed
                    # only 2/3 into each block; each qn head evacs right after
                    # its block closes
                    def qblock(outs, cols):
                        for ti, term in enumerate((0, 2, 1)):
                            w_t = wq8 if term in (0, 1) else wqr
                            h_t = hidr2[:] if term == 1 else hid8[:, :, sl]
                            for kg in range(KG):
                                st_ = (ti == 0 and kg == 0)
                                sp_ = (ti == 2 and kg == KG - 1)
                                kk = slice(2 * kg, 2 * kg + 2)
                                for out, cb in zip(outs, cols):
                                    nc.tensor.matmul(out[:], w_t[:, kk, cb],
                                                     h_t[:, kk, :], start=st_,
                                                     stop=sp_, perf_mode=DRow)
                    qblock(qa_ps, [slice(512 + p * 128, 512 + (p + 1) * 128)
                                   for p in range(2)])
                    for p in range(2):
                        # q rope: qc = qa*cos; qr = rotate_half(qa)*sin with the
                        # sign flip folded into an stt (cross-partition reads)
                        qc = p2.tile([128, CH], bf16, tag="qc", bufs=1, name="qc")
                        qr = p2.tile([128, CH], bf16, tag="qr", bufs=1, name="qr")
                        for hh in (0, 64):
                            nc.vector.tensor_mul(qc[hh:hh + 64, :], qa_ps[p][hh:hh + 64, :],
                                                 cs2[0:64, sl])
                            nc.vector.scalar_tensor_tensor(qr[hh:hh + 32, :],
                                                           qa_ps[p][hh + 32:hh + 64, :],
                                                           cm1[64:96, :], cs2[64:96, sl],
                                                           op0=mult, op1=mult)
                            nc.vector.tensor_mul(qr[hh + 32:hh + 64, :],
                                                 qa_ps[p][hh:hh + 32, :], cs2[96:128, sl])
                        for i, hh in ((0, 0), (1, 64)):
                            h2 = 2 * p + i
                            nc.vector.tensor_add(qpr16[0:32, h2, :],
                                                 qc[hh:hh + 32, :], qr[hh:hh + 32, :])
                            nc.vector.tensor_add(qpr16[32:64, h2, :],
                                                 qc[hh + 32:hh + 64, :], qr[hh + 32:hh + 64, :])
                    # rope q in fp8: plane4 = [qp8; qp_res], plane5 rows 0:64 =
                    # qp8 again (pairs with the k-side residual)
                    for h in range(HL):
                        nc.scalar.activation(ql8a[0:64, 4, h, :], qpr16[:, h, :],
                                             Copy, scale=1.0)
                        nc.vector.scalar_tensor_tensor(ql8a[64:128, 4, h, :],
                                                       qpr16[:, h, :], c_one[0:64, :],
                                                       ql8a[0:64, 4, h, :],
                                                       op0=mult, op1=subtract)
                        nc.scalar.activation(ql8a[0:64, 5, h, :], qpr16[:, h, :],
                                             Copy, scale=1.0)
                    for h in range(HL):
                        qblock([qn_ps[h]], [slice(h * 128, (h + 1) * 128)])
                        t = p2.tile([128, CH], fp16, tag="qn16", bufs=4, name=f"qn16_{h}")
                        nc.scalar.activation(t[:], qn_ps[h][:], Copy, scale=0.125)
                        qn16.append(t)

                    # previous chunk's o_proj: one ht-quarter is emitted
                    # after each head below, peppering the PE queue so o_proj
                    # matmuls fill the attention pipeline bubbles
                    oproj_quarters = prev_oproj if prev_oproj is not None else []
                    prev_oproj = None

                    vo8a = p2.tile([128, HL, CH], fp8, tag="vo8a", bufs=2, name="vo8a")
                    vor8 = p2.tile([128, HL, CH], fp8, tag="vor8", bufs=2, name="vor8")
                    if oproj_quarters:
                        # quarter 0 fills head 0's ql/score warmup bubble
                        oproj_quarters[0]()
                    prev_tail = None
                    for h in range(HL):
                        # q_lat^T[c, s]: plain fp8 matmuls (K=128), evac x 2^-2
                        for ci in range(4):
                            ql_ps = ps_tile("ql_ps")
                            nc.tensor.matmul(ql_ps[:], kb16[:, h, ci * 128:(ci + 1) * 128],
                                             qn16[h][:], start=True, stop=True)
                            if ci % 2 == 0:
                                nc.scalar.activation(ql8a[:, ci, h, :], ql_ps[:], Copy,
                                                     scale=1.0)
                            else:
                                nc.vector.tensor_scalar_mul(ql8a[:, ci, h, :], ql_ps[:], 1.0)

                        # emit the previous head's tail now so its psum-freeing
                        # chain overlaps this head's ql/score matmuls
                        if prev_tail is not None:
                            prev_tail()
                            prev_tail = None

                        # t-pair order: diagonal pairs first, then history pairs
                        prs = [(2 * j, (0, 0), True), (2 * j + 1, (256, 384), True)] + \
                              [(m, (0, 0), False) for m in range(0, 2 * j)]

                        def do_pair(m, sts, diag):
                            e8p = None if diag else p2.tile([128, 2, CH], fp8,
                                                            tag="e8p", bufs=5, name="e8p")
                            exs = []
                            for par in range(2):
                                st = sts[par]
                                t_i = 2 * m + par
                                tb = slice(t_i * 128, (t_i + 1) * 128)
                                sc_ps = ps_tile("sc_ps")
                                nc.tensor.matmul(sc_ps[:, st:], ckvT8[:, 4:6, tb],
                                                 ql8a[:, 4:6, h, st:],
                                                 start=True, stop=False, perf_mode=DRow)
                                if diag:
                                    # mask add as a tiny fp16 identity-matmul on
                                    # the PE, folded into the score accumulation
                                    # (inputs are resident, so it can issue
                                    # while the ql8a latent evacs land)
                                    kd = t_i - 4 * j
                                    ma, mb2 = ((0, 128), (0, 256),
                                               (256, 384), (384, 512))[kd]
                                    nc.tensor.matmul(sc_ps[:, ma:mb2], idm8[:],
                                                     masks[:, kd, ma:mb2],
                                                     start=False, stop=False)
                                nc.tensor.matmul(sc_ps[:, st:], ckvT8[:, 0:2, tb],
                                                 ql8a[:, 0:2, h, st:],
                                                 start=False, stop=False, perf_mode=DRow)
                                nc.tensor.matmul(sc_ps[:, st:], ckvT8[:, 2:4, tb],
                                                 ql8a[:, 2:4, h, st:],
                                                 start=False, stop=True, perf_mode=DRow)
                                ex16 = p2.tile([128, CH], fp16, tag="ex16", bufs=6, name="ex16")
                                nc.scalar.activation(ex16[:, st:], sc_ps[:, st:], Exp,
                                                     bias=zb128[:], scale=SCALE / WS)
                                if e8p is not None:
                                    nc.vector.tensor_copy(e8p[:, par, :], ex16[:])
                                exs.append(ex16)
                            return tuple(exs) + (e8p,)

                        vo_box = []
                        rs_box = []

                        def pv(idx, m, sts, ex_a, ex_b, e8p):
                            first, last = (idx == 0), (idx == len(prs) - 1)
                            for par, ext in ((0, ex_a), (1, ex_b)):
                                st = sts[par]
                                t_i = 2 * m + par
                                nc.tensor.matmul(vo_box[0][:, st:],
                                                 vT16[:, h, t_i, :], ext[:, st:],
                                                 start=(first and par == 0),
                                                 stop=(last and par == 1))
                                if e8p is None:
                                    nc.tensor.matmul(rs_box[0][:, st:], on16[:], ext[:, st:],
                                                     start=(first and par == 0),
                                                     stop=(last and par == 1))
                            if e8p is not None:
                                nc.tensor.matmul(rs_box[0][:, :], on8[:], e8p[:],
                                                 start=first, stop=last, perf_mode=DRow)

                        pend = []
                        for idx, (m, sts, diag) in enumerate(prs):
                            pair_t = do_pair(m, sts, diag)
                            if idx == 0:
                                vo_box.append(ps_tile("vo_ps"))
                                rs_box.append(ps_tile("rs_ps"))
                            pend.append((idx, m, sts) + pair_t)
                            if len(pend) > 4:
                                pv(*pend.pop(0))
                        for pd in pend:
                            pv(*pd)

                        def make_tail(h, vo_ps, rs_ps):
                            def tail():
                                # softmax denominator: full-row reciprocal on DVE
                                rbc = p2.tile([128, CH], f32r, tag="rbc", bufs=1, name="rbc")
                                nc.vector.reciprocal(rbc[:], rs_ps[:])
                                # normalize v-out, fp8 + residual split
                                tmp16 = p2.tile([128, CH], fp16, tag="tmp16", bufs=2, name="tmp16")
                                nc.vector.scalar_tensor_tensor(tmp16[:], vo_ps[:],
                                                               c_one[:], rbc[:],
                                                               op0=mult, op1=mult)
                                nc.gpsimd.tensor_copy(vo8a[:, h, :], tmp16[:])
                                nc.vector.scalar_tensor_tensor(vor8[:, h, :], tmp16[:],
                                                               c_one[:], vo8a[:, h, :],
                                                               op0=mult, op1=subtract)
                            return tail

                        prev_tail = make_tail(h, vo_box[0], rs_box[0])
                        if h + 1 < len(oproj_quarters):
                            oproj_quarters[h + 1]()
                    prev_tail()

                    # o_proj partial (3-term fp8x2): out^T = sum_h wo^T.T @ v_out^T
                    def make_oproj(sl, vo8a, vor8):
                        def oproj(hts):
                            for ht in hts:
                                htb = slice(ht * 128, (ht + 1) * 128)
                                oo_ps = ps_tile("oo_ps")
                                for g2 in range(2):
                                    hh2 = slice(2 * g2, 2 * g2 + 2)
                                    nc.tensor.matmul(oo_ps[:], wo8[:, hh2, htb], vo8a[:, hh2, :],
                                                     start=(g2 == 0), stop=False, perf_mode=DRow)
                                    nc.tensor.matmul(oo_ps[:], wo8[:, hh2, htb], vor8[:, hh2, :],
                                                     start=False, stop=False, perf_mode=DRow)
                                    nc.tensor.matmul(oo_ps[:], wor[:, hh2, htb], vo8a[:, hh2, :],
                                                     start=False, stop=(g2 == 1), perf_mode=DRow)
                                oo_sb = p2.tile([128, CH], fp16, tag="oo_sb", bufs=3, name="oo_sb")
                                if ht % 2 == 0:
                                    nc.scalar.activation(oo_sb[:], oo_ps[:], Copy,
                                                         scale=1.0 / (WS * WS))
                                else:
                                    nc.vector.tensor_scalar_mul(oo_sb[:], oo_ps[:],
                                                                1.0 / (WS * WS))
                                if ht % 3 == 0:
                                    nc.sync.dma_start(out_d[htb, sl], oo_sb[:])
                                elif ht % 3 == 1:
                                    nc.scalar.dma_start(out_d[htb, sl], oo_sb[:])
                                else:
                                    nc.gpsimd.dma_start(out_d[htb, sl], oo_sb[:])
                        return oproj

                    _op = make_oproj(sl, vo8a, vor8)
                    prev_oproj = [
                        (lambda q=q, f=_op: f(range(4 * q, 4 * q + 4)))
                        for q in range(4)]
                for q in prev_oproj:
                    q()

    nc.compile()
    return nc


# ---------------------------------------------------------------------------
# host-side input prep / output assembly
# ---------------------------------------------------------------------------
_PERM = np.concatenate([np.arange(0, DR, 2), np.arange(1, DR, 2)])


def _rope_tables(pos, s):
    inv_freq = 1.0 / (THETA ** (np.arange(0, DR, 2, dtype=np.float64) / DR))
    t = pos.astype(np.float64)
    freqs = t[:, None] * inv_freq
    emb = np.concatenate([freqs, freqs], axis=-1)          # [s, DR]
    cosT = np.cos(emb).T.astype(np.float32)                # [DR, s]
    sinT = np.sin(emb).T.astype(np.float32)
    return cosT, sinT


def _masks():
    t = np.arange(128)[:, None]
    c = np.arange(CH)[None, :]
    m = np.zeros((128, 4, CH), np.float32)
    for kd in range(4):
        m[:, kd, :] = np.where(c >= 128 * kd + t, 0.0, -30000.0).astype(np.float32)
    return m


def _fp8_split(x):
    a = x.astype(FP8)
    r = (x - a.astype(np.float32)).astype(FP8)
    return a, r


def prep_core_inputs(inputs, core, s=S, hid=HID):
    b, g = core // 4, core % 4
    heads = slice(HL * g, HL * (g + 1))
    hs = np.asarray(inputs["hidden_states"], np.float32)[b, :s, :hid]
    m = {}
    m["hid8"], m["hidr8"] = _fp8_split(np.ascontiguousarray(hs.T))

    wq = np.asarray(inputs["q_nope_weight"], np.float32).reshape(H, DN, HID)[heads, :, :hid]
    wq_t = wq.transpose(2, 0, 1).reshape(hid, HL * DN)
    wqp = np.asarray(inputs["q_pe_weight"], np.float32).reshape(H, DR, HID)[heads, :, :hid]
    a = wqp[:, _PERM, :]                                   # [4, 64, hid]
    A = a.reshape(2, 128, hid)
    wqpe_t = np.concatenate([A[0], A[1]], axis=0).T
    m["wq8"], m["wqr8"] = _fp8_split(np.concatenate([wq_t, wqpe_t], axis=1) * WS)

    wkv = np.asarray(inputs["kv_a_weight"], np.float32)[:, :hid]
    kpe_a = wkv[KVR:][_PERM]
    kpe_b = np.concatenate([-kpe_a[32:], kpe_a[:32]], axis=0)
    wkv_t = np.ascontiguousarray(
        np.concatenate([wkv[:KVR], kpe_a, kpe_b], axis=0).T * WS)
    m["wkv8"], m["wkvr8"] = _fp8_split(wkv_t)

    m["ln_t"] = np.ascontiguousarray(
        np.asarray(inputs["kv_a_ln_weight"], np.float32).reshape(4, 128).T)
    m["kb16"] = (np.asarray(inputs["k_b_weight"], np.float32)[heads] * (WS / 4)).astype(np.float16)
    m["vb16_t"] = np.ascontiguousarray(
        np.asarray(inputs["v_b_weight"], np.float32)[heads].transpose(0, 2, 1) * WS
    ).astype(np.float16)
    wo_t = np.ascontiguousarray(
        np.asarray(inputs["o_weight"], np.float32)[:hid, HL * DV * g:HL * DV * (g + 1)].T * WS)
    m["wo8_t"], m["wor8_t"] = _fp8_split(wo_t)

    pos = np.asarray(inputs["position_ids"]).reshape(-1)[:s]
    cosT, sinT = _rope_tables(pos, s)                      # [64, s] each
    m["cs2"] = np.ascontiguousarray(np.vstack([cosT, sinT])).astype(BF16)
    m["ckrope"] = np.ascontiguousarray(
        np.vstack([cosT, sinT]) * (1.0 / WS)).astype(BF16)
    m["masks"] = (_masks() / 128.0).astype(FP8)
    m["identm8"] = (np.eye(128, dtype=np.float32) * 128.0).astype(FP8)
    m["ones_sq"] = np.ones((128, 128), np.float32)
    m["ones16"] = np.ones((128, 128), np.float16)
    m["ones8"] = np.ones((128, 2, 128), np.float32).astype(FP8)
    return m


_NC_CACHE = {}


def _get_nc():
    if "nc" not in _NC_CACHE:
        _NC_CACHE["nc"] = build_nc()
    return _NC_CACHE["nc"]


def kernel(**inputs):
    from concourse import bass_utils

    nc = _get_nc()
    in_maps = [prep_core_inputs(inputs, c) for c in range(NCORES)]
    res = bass_utils.run_bass_kernel_spmd(nc, in_maps, core_ids=list(range(NCORES)))
    out = np.empty((B, S, HID), np.float32)
    for b in range(B):
        acc = np.array(res.results[4 * b]["out_t"], np.float32)
        for g in range(1, 4):
            acc += res.results[4 * b + g]["out_t"]
        out[b] = acc.T
    return out

